# revision 72
# baseline (speedup 1.0000x reference)
"""Trainium2 Bass kernel for nn_CIE_89893665505337 (gnn_message_passing).

2x [MHA(global over 4096 nodes) + FF] transformer blocks + HypergraphConv.
8-core SPMD: nodes sharded 512/core, transposed activations hT [D=128, n],
fp16 matmul operands, f32 residual stream, AllGather for K/V, xt, e.
"""
import os
import sys

for _p in ("/opt/trn_rl_repo", "/root/.axon_site/_ro/trn_rl_repo"):
    if os.path.isdir(_p) and _p not in sys.path:
        sys.path.insert(0, _p)

import numpy as np

import concourse.bacc as bacc
import concourse.bass as bass
import concourse.tile as tile
from concourse import mybir
from concourse.bass_utils import run_bass_kernel_spmd

F32 = mybir.dt.float32
F16 = mybir.dt.float16
I32 = mybir.dt.int32
I8 = mybir.dt.int8
# exp engine split (Act / DVE only: gpsimd has no PSUM port); 32 ops/layer
EXPSPLIT = "AD"
AF = mybir.ActivationFunctionType
ALU = mybir.AluOpType

W = 8            # cores
N = 4096         # nodes
D = 128          # model dim
H = 2            # heads
DH = 64          # head dim
FF = 256         # ff dim
NE = 2048        # hyperedges
NL = N // W      # 512 local nodes
EL = NE // W     # 256 local hyperedges
EPS = 1e-5
NCH = N // 128   # 32 m-chunks
RSQRT_MAGIC = 0x5F3759DF

H_SZ = 128 * NL            # fp16 elems of hT_loc [128, 512]


def build_program():
    nc = bacc.Bacc("TRN2", target_bir_lowering=False, debug=False, num_devices=W)

    def inp(name, shape, dt=F32):
        return nc.dram_tensor(name, shape, dt, kind="ExternalInput")

    F8 = mybir.dt.float8e4
    # per-core inputs
    xT32 = inp("xT32", [128, NL])
    xT16 = inp("xT16", [128, NL], F16)
    xTf8 = inp("xTf8", [128, W, NL], F8)      # full x^T, rank-blocked, fp8
    # packed weights (one DMA each):
    # CW16: Wkv 0:258 | Wq 258:386 | Wo 386:514 | W1G 514:770 | Wh 770:898
    #       | id16 898:1026 | w2 1026:1282 | ones/128 1282:1283
    CW16 = inp("CW16", [128, 1283], F16)
    # CW32: bkv 0:131 | bq | bo | bf2 | bh
    CW32 = inp("CW32", [128, 135])
    # CROW: ln1r 0:768 | ln2r 768:1536 | w2sum 1536:1664 | ln1rn 1664:1792
    #       | ln2rn 1792:1920 | g1n 1920:2176 | fb1 2176:2432
    CROW = inp("CROW", [1, 2562], F16)
    # conv operand: Mt[p, c, n] = S[loc(c*128+p), n] * Dinv[n]  (fp16)
    Mt = inp("Mt", [128, 4, N], F16)

    out_t = nc.dram_tensor("outT", [128, NL], F32, kind="ExternalOutput")

    # AG bounce tensors (internal DRAM); outputs Shared
    kv_in = nc.dram_tensor("kv_in1", [H_SZ], F8)
    kv_out = nc.dram_tensor("kv_out1", [W, H_SZ], F8, addr_space="Shared")
    # conv ReduceScatter bounce
    rs_in = nc.dram_tensor("rs_in", [W, 128, NL], F16)
    rs_out = nc.dram_tensor("rs_out", [128, NL], F16)

    RG = [list(range(W))]

    with tile.TileContext(nc) as tc:
        with (
            tc.tile_pool(name="wpool", bufs=1) as wp,      # persistent weights/consts
            tc.tile_pool(name="sb", bufs=3) as sb,         # general sbuf tiles
            tc.tile_pool(name="kv", bufs=2) as kvp,        # kT/V per layer
            tc.tile_pool(name="expp", bufs=4) as expp,     # exp tiles
            tc.tile_pool(name="hp", bufs=1) as hp,         # conv H tiles
            tc.tile_pool(name="ps_s", bufs=2, space="PSUM") as ps_s,   # scores 2 banks each
            tc.tile_pool(name="ps_o", bufs=2, space="PSUM") as ps_o,   # attn out acc
            tc.tile_pool(name="ps_m", bufs=2, space="PSUM") as ps_m,   # misc
        ):
            # ---- load constants ----
            def load(name, shape, dram, dt=F32):
                t = wp.tile(shape, dt, name=name)
                nc.sync.dma_start(t[:], dram[:])
                return t

            F8 = mybir.dt.float8e4
            # critical path first: packed weights, then layer-1 inputs
            c_w16 = wp.tile([128, 1283], F16, name="c_w16")
            nc.gpsimd.dma_start(c_w16[:], CW16[:])
            c_w32 = wp.tile([128, 135], F32, name="c_w32")
            nc.gpsimd.dma_start(c_w32[:], CW32[:])
            hT16 = sb.tile([128, NL], F16, tag="hT16")
            nc.gpsimd.dma_start(hT16[:], xT16[:])
            c_row = wp.tile([1, 2562], F16, name="c_row")
            nc.gpsimd.dma_start(c_row[:], CROW[:])
            w_kv = c_w16
            w_k = c_w16[:, 0:128]
            w_v = c_w16[:, 128:256]
            w_q = c_w16[:, 258:386]
            w_o = c_w16[:, 386:514]
            w_1g = c_w16[:, 514:770]
            w_h = c_w16[:, 770:898]
            c_id16 = c_w16[:, 898:1026]
            c_one16 = c_w16[:, 1282:1283]
            c_bk = c_w32[:, 0:1]
            c_bq = c_w32[:, 131:132]
            c_bo = c_w32[:, 132:133]
            c_bf2 = c_w32[:, 133:134]
            c_bh = c_w32[:, 134:135]

            def w2sl(cix):
                return c_w16[:, 1026 + 128 * cix:1026 + 128 * (cix + 1)]

            # layer-1 full-x load: highest DMA priority (attention-1 gate)
            hTf0 = kvp.tile([128, W, NL], F8, tag="hT_full", name="hTf0")
            nc.sync.dma_start(hTf0[:, 0:4, :], xTf8[:, 0:4, :])
            nc.sync.dma_start(hTf0[:, 4:8, :], xTf8[:, 4:8, :])

            c_ln = [c_row[0:1, 0:768], c_row[0:1, 768:1536]]
            c_w2s = c_row[0:1, 1536:1664]
            c_lnn = [c_row[0:1, 1664:1792], c_row[0:1, 1792:1920]]
            r_g1n = c_row[0:1, 1920:2176]
            r_fb1 = c_row[0:1, 2176:2432]
            r_bv = c_row[0:1, 2432:2560]
            ones_r = c_row[0:1, 256:384]
            c_magic = wp.tile([128, 4], I32)
            nc.vector.memset(c_magic[:], RSQRT_MAGIC)
            c_ones1 = wp.tile([1, 64], F16)   # lhsT for den broadcast
            nc.vector.memset(c_ones1[:], 1.0)
            # warm the Act table set before the first real Act op needs it
            warm = wp.tile([1, 16], F32, name="warm")
            nc.vector.memset(warm[:], 0.0)
            nc.scalar.activation(warm[:], warm[:], AF.Exp)

            hT32 = sb.tile([128, NL], F32, tag="hT32")
            nc.sync.dma_start(hT32[:], xT32[:])

            # conv M operand: DMAs ride layer-2's AllGather window (idle DMA)
            m_t = hp.tile([128, 4, N], F16, name="m_t")

            def prefetch_conv_h(li, gate=None):
                if li == 1:
                    if gate is not None:
                        with nc.allow_low_precision(reason="dma gate"):
                            nc.vector.tensor_copy(m_t[0:1, 0:4, 0:1], gate)
                    for c in range(4):
                        nc.sync.dma_start(m_t[:, c, :], Mt[:, c, :])

            # ---------- helpers ----------
            def layer_norm(t32, t16_out, lnc, lncn, ff=False, t8_out=None,
                           t16_fast=False):
                """t32: [128, NL] f32 pre-LN input. Returns (h_ln f32, T strips,
                t16 fp16 copy of t32). T rows: [inv | m*inv] (+ [m | sigma]
                when ff=True, for the folded FF entry). Optionally writes an
                fp16 copy of the OUTPUT into t16_out."""
                t16 = sb.tile([128, NL], F16, tag="ln_t16")
                nc.vector.tensor_copy(t16[:], t32[:])
                t2 = sb.tile([128, NL], F16, tag="ln_t2")
                with nc.allow_low_precision(reason="LN sq to fp16"):
                    nc.scalar.square(t2[:], t32[:])
                stats = ps_m.tile([128, 8], F32, tag="m")
                for s in range(4):
                    nc.tensor.matmul(stats[:, s:s + 1],
                                     t16[:, s * 128:(s + 1) * 128], c_one16[:])
                    nc.tensor.matmul(stats[:, 4 + s:5 + s],
                                     t2[:, s * 128:(s + 1) * 128], c_one16[:])
                # stats already means (ones col = 1/128)
                m = stats[:, 0:4]
                msq = sb.tile([128, 4], F32, tag="ln_msq")
                nc.scalar.square(msq[:], m)
                ve = sb.tile([128, 4], F32, tag="ln_ve")
                nc.vector.scalar_tensor_tensor(ve[:], stats[:, 4:8], EPS, msq[:],
                                               ALU.add, ALU.subtract)
                # rsqrt via bit trick + 1 Newton iter (~0.2% rel err)
                sh = sb.tile([128, 4], I32, tag="ln_sh")
                nc.vector.tensor_scalar(sh[:], ve[:].bitcast(I32), 1, None,
                                        ALU.logical_shift_right)
                P = sb.tile([128, 8], F32, tag="ln_P")
                y = P[:, 0:4]
                nc.vector.tensor_tensor(y.bitcast(I32), c_magic[:], sh[:], ALU.subtract)
                a = sb.tile([128, 4], F32, tag="ln_a")
                nc.vector.tensor_tensor(a[:], y, y, ALU.mult)
                nc.vector.scalar_tensor_tensor(a[:], a[:], -0.5, ve[:],
                                               ALU.mult, ALU.mult)
                nc.vector.scalar_tensor_tensor(y, a[:], 1.5, y,
                                               ALU.add, ALU.mult)
                nc.vector.tensor_tensor(P[:, 4:8], m, y, ALU.mult)  # m*inv
                # strips to partition 0 via M=1 matmuls against identity:
                # out[0, j] = P16[j, s]
                nq = 4 if ff else 2
                P16 = sb.tile([128, 16], F16, tag="ln_P16")
                nc.vector.tensor_copy(P16[:, 0:8], P[:])
                if ff:
                    with nc.allow_low_precision(reason="LN strips fp16"):
                        nc.scalar.copy(P16[:, 8:12], m)
                        nc.vector.tensor_tensor(P16[:, 12:16], ve[:], y, ALU.mult)
                psT = [ps_s.tile([1, NL], F32, tag="scr", name=f"psT{q}")
                       for q in range(nq)]
                for q in range(nq):
                    for s in range(4):
                        nc.tensor.matmul(psT[q][:, s * 128:(s + 1) * 128],
                                         P16[:, 4 * q + s:4 * q + s + 1],
                                         c_id16[:])
                T = sb.tile([1, 2048], F16, tag="ln_T")
                with nc.allow_low_precision(reason="LN strip fp16"):
                    nc.scalar.copy(T[0:1, 0:NL], psT[0][:])
                    nc.vector.tensor_copy(T[0:1, NL:2 * NL], psT[1][:])
                    if ff:
                        nc.scalar.copy(T[0:1, 2 * NL:3 * NL], psT[2][:])
                        nc.vector.tensor_copy(T[0:1, 3 * NL:4 * NL], psT[3][:])
                # broadcast matmuls: A = g (x) inv ; B = (-g) (x) (m*inv) + b (x) 1
                psA = ps_m.tile([128, NL], F32, tag="m")
                psB = ps_m.tile([128, NL], F32, tag="m")
                nc.tensor.matmul(psA[:], lnc[0:1, 0:128], T[0:1, 0:NL])
                nc.tensor.matmul(psB[:], lncn[0:1, :], T[0:1, NL:2 * NL],
                                 start=True, stop=False)
                nc.tensor.matmul(psB[:], lnc[0:1, 128:256], lnc[0:1, 256:768],
                                 start=False, stop=True)
                u = sb.tile([128, NL], F32, tag="ln_u")
                nc.vector.tensor_tensor(u[:], t32[:], psA[:], ALU.mult)
                h_ln = sb.tile([128, NL], F32, tag="ln_out")
                if t8_out is not None:
                    # AG gate: fp8 output written first, directly from the add
                    with nc.allow_low_precision(reason="h8 for AllGather"):
                        nc.vector.tensor_tensor(t8_out[:], u[:], psB[:], ALU.add)
                nc.vector.tensor_tensor(h_ln[:], u[:], psB[:], ALU.add)
                if t16_out is not None:
                    with nc.allow_low_precision(reason="h16 copy"):
                        if t16_fast:
                            nc.vector.tensor_copy(t16_out[:], h_ln[:])
                        else:
                            nc.gpsimd.tensor_copy(t16_out[:], h_ln[:])
                return h_ln, T, t16

            # ---------- transformer layer ----------
            def mha_ff_layer(li, hT32_in, hT16_in, hT8_in, hTf_pre):
                # full h: layer 1 gets it free from the host input (pre-loaded);
                # layer 2 AllGathers the fp8 h produced by layer 1
                if hTf_pre is not None:
                    hTf = hTf_pre
                else:
                    hTf = kvp.tile([128, W, NL], F8, tag="hT_full")
                    nc.gpsimd.dma_start(
                        kv_in[:].rearrange("(p j) -> p j", p=128), hT8_in[:])
                    nc.gpsimd.collective_compute(
                        "AllGather", ALU.bypass, replica_groups=RG,
                        ins=[kv_in[:]], outs=[kv_out[:]])

                ps_q = ps_m.tile([128, NL], F32, tag="m")
                nc.tensor.matmul(ps_q[:], w_q[:], hT16_in[:])
                # qz: fp8 q with a zero slot; scores use DoubleRow fp8 where
                # the k-side second slot is garbage annihilated by the zeros
                qz = sb.tile([128, 2, NL], F8, tag="qz")
                nc.gpsimd.memset(qz[:, 1, :], 0.0)
                with nc.allow_low_precision(reason="fp8 attention"):
                    nc.vector.tensor_scalar(qz[:, 0, :], ps_q[:], c_bq[:],
                                            None, ALU.add)

                # unpack h_full, then recompute kT/V locally (pipelines with attn)
                if hTf_pre is None:
                    kvv = kv_out[:].rearrange("r (p j) -> p r j", p=128)
                    nc.sync.dma_start(hTf[:, 0:4, :], kvv[:, 0:4, :])
                    nc.sync.dma_start(hTf[:, 4:8, :], kvv[:, 4:8, :])
                # kT8 layout [128, 5, 128]/rank: slots 0-3 = m-chunks, slot 4 =
                # pad so scores can view [64, 2, 128] slot pairs (cc, cc+1)
                kT8 = kvp.tile([128, W, 5, 128], F8, tag="kT_full")
                # per-head v padded to M=96: cols 0:64 v, 64 ones (denom),
                # 65:96 zero (DoubleRow needs M % 32 == 0, contiguous pairs,
                # dst partition base 0)
                vh = [kvp.tile([128, NCH, 96], F8, tag=f"v_h{h}",
                               name=f"vh{li}_{h}")
                      for h in range(H)]
                for h in range(H):
                    nc.gpsimd.memset(vh[h][:, :, 65:96], 0.0)
                    nc.gpsimd.memset(vh[h][:, :, 64:65], 1.0)
                # pad slot must be finite (NaN * 0 would poison DoubleRow)
                nc.gpsimd.memset(kT8[:, :, 4, :], 0.0)
                with nc.allow_low_precision(reason="fp8 attention"):
                    for r in range(W):
                        ps_k = ps_m.tile([128, 4, 128], F32, tag="m")
                        nc.tensor.matmul(ps_k[:], w_k[:], hTf[:, r, :])
                        if r % 2 == 0:
                            nc.scalar.activation(kT8[:, r, 0:4, :], ps_k[:],
                                                 AF.Identity, bias=c_bk[:])
                        else:
                            nc.vector.tensor_scalar(kT8[:, r, 0:4, :], ps_k[:],
                                                    c_bk[:], None, ALU.add)
                        for cp in range(2):
                            pv = ps_m.tile([128, 2, 128], F32, tag="m")
                            for j in range(2):
                                cc = 2 * cp + j
                                # bias prefilled via PE so the drain is a copy
                                nc.tensor.matmul(pv[:, j, :], ones_r[:],
                                                 r_bv[:], start=True, stop=False)
                                nc.tensor.matmul(
                                    pv[:, j, :],
                                    hTf[:, r, cc * 128:(cc + 1) * 128], w_v[:],
                                    start=False, stop=True)
                            c = 4 * r + 2 * cp
                            nc.vector.tensor_copy(vh[0][:, c:c + 2, 0:64],
                                                  pv[:, :, 0:64])
                            nc.scalar.copy(vh[1][:, c:c + 2, 0:64],
                                           pv[:, :, 64:128])

                prefetch_conv_h(li, gate=kT8[0:1, 0:1, 0:4, 0:1])

                # attention: fp8 DoubleRow scores + AV; exp split Act/DVE/Pool
                DR = mybir.MatmulPerfMode.DoubleRow
                S1, S2 = 1.4426950408889634, 54.76  # fp8e4 Schraudolph exp
                oT = sb.tile([128, NL], F16, tag="oT")
                po = [ps_o.tile([96, NL], F32, tag="o_acc", name=f"po{li}_{h}")
                      for h in range(H)]
                expi = 0
                for g in range(NCH // 2):
                    # ex_g[:, h, j, :]: head-major so AV reads a contiguous
                    # [128, 2, 512] pair per head; one exp op per chunk j
                    # covers both heads (strided output)
                    ex_g = expp.tile([128, H, 2, NL], I8, tag="exp",
                                     name=f"ex{li}_{g}")
                    for j in range(2):
                        c = 2 * g + j
                        r, cc = c // 4, c % 4
                        psc = ps_s.tile([128, H, NL], F32, tag="scr",
                                        name=f"scr{li}_{c}")
                        for h in range(H):
                            hs = slice(h * 64, (h + 1) * 64)
                            nc.tensor.matmul(
                                psc[:, h, :],
                                kT8[hs, r, cc:cc + 2, :],
                                qz[hs, :, :], perf_mode=DR)
                        eng = EXPSPLIT[expi % len(EXPSPLIT)]
                        expi += 1
                        with nc.allow_low_precision(reason="fp8 exp"):
                            if eng == "A":
                                nc.scalar.activation(
                                    ex_g[:, :, j, :].bitcast(F8), psc[:],
                                    AF.Exp, scale=0.125)
                            else:
                                nc.vector.tensor_scalar(ex_g[:, :, j, :],
                                                        psc[:], S1, S2,
                                                        ALU.mult, ALU.add)
                    for h in range(H):
                        exr = ex_g[:, h, :, :].bitcast(F8)
                        nc.tensor.matmul(
                            po[h][:], vh[h][:, 2 * g:2 * g + 2, :], exr,
                            start=(g == 0), stop=(g == NCH // 2 - 1),
                            perf_mode=DR)
                for h in range(H):
                    hs = slice(h * 64, (h + 1) * 64)
                    # normalize: fast recip of den row, broadcast, multiply
                    den32 = sb.tile([1, NL], F32, tag="den32")
                    nc.scalar.copy(den32[:], po[h][64:65, :])
                    rden32 = sb.tile([1, NL], F32, tag="rden32")
                    nc.vector.reciprocal_approx_fast(rden32[:], den32[:])
                    rden = sb.tile([1, NL], F16, tag="rden")
                    with nc.allow_low_precision(reason="attn denom recip fp16"):
                        nc.scalar.copy(rden[:], rden32[:])
                    pden = ps_m.tile([64, NL], F32, tag="m")
                    nc.tensor.matmul(pden[:], c_ones1[:], rden[:])
                    denB = sb.tile([64, NL], F16, tag="denB")
                    with nc.allow_low_precision(reason="denB fp16"):
                        nc.scalar.copy(denB[:], pden[:])
                    nc.vector.tensor_tensor(oT[hs, :], po[h][0:64, :], denB[:],
                                            ALU.mult)

                # o-projection + residual
                ps_p = ps_m.tile([128, NL], F32, tag="m")
                nc.tensor.matmul(ps_p[:], w_o[:], oT[:])
                t1 = sb.tile([128, NL], F32, tag="resid1")
                nc.vector.scalar_tensor_tensor(t1[:], ps_p[:], c_bo[:], hT32_in[:],
                                               ALU.add, ALU.add)

                h1_32, T1, t1_16 = layer_norm(t1, None, c_ln[0], c_lnn[0],
                                              ff=True)

                # FF folded through LN1: tanh arg = inv (.) [W1G^T t1
                # + G1N (x) m + FB1 (x) sigma]; sigmoid affine folded into
                # host-scaled W2 (0.5*W2) plus const row 0.5*colsum(W2)
                psI = ps_s.tile([128, NL], F32, tag="scr", name="psI")
                nc.tensor.matmul(psI[:], c_ln[0][0:1, 256:384], T1[0:1, 0:NL])
                invB = sb.tile([128, NL], F16, tag="invB")
                with nc.allow_low_precision(reason="invB fp16"):
                    nc.scalar.copy(invB[:], psI[:])
                z = sb.tile([128, 2, NL], F16, tag="z")
                zarg = sb.tile([128, 2, NL], F16, tag="zarg")
                for f in range(2):
                    pz = ps_m.tile([128, NL], F32, tag="m")
                    nc.tensor.matmul(pz[:], w_1g[:, f * 128:(f + 1) * 128],
                                     t1_16[:], start=True, stop=False)
                    nc.tensor.matmul(pz[:], r_g1n[0:1, f * 128:(f + 1) * 128],
                                     T1[0:1, 2 * NL:3 * NL],
                                     start=False, stop=False)
                    nc.tensor.matmul(pz[:], r_fb1[0:1, f * 128:(f + 1) * 128],
                                     T1[0:1, 3 * NL:4 * NL],
                                     start=False, stop=True)
                    with nc.allow_low_precision(reason="tanh arg fp16"):
                        nc.vector.tensor_tensor(zarg[:, f, :], pz[:], invB[:],
                                                ALU.mult)
                ps_f = ps_m.tile([128, NL], F32, tag="m")
                nc.tensor.matmul(ps_f[:], c_w2s[:], c_ln[0][0:1, 256:768],
                                 start=True, stop=False)
                for f in range(2):
                    nc.scalar.activation(z[:, f, :], zarg[:, f, :], AF.Tanh)
                    nc.tensor.matmul(ps_f[:], w2sl(f), z[:, f, :],
                                     start=False, stop=(f == 1))
                t2 = sb.tile([128, NL], F32, tag="resid2")
                nc.vector.scalar_tensor_tensor(t2[:], ps_f[:], c_bf2[:], h1_32[:],
                                               ALU.add, ALU.add)

                h2_16 = sb.tile([128, NL], F16, tag="hT16")
                h2_8 = None
                if li == 0:
                    h2_8 = sb.tile([128, NL], F8, tag="hT8")
                h2_32, _, _ = layer_norm(t2, h2_16, c_ln[1], c_lnn[1],
                                         t8_out=h2_8, t16_fast=(li == 1))
                return h2_32, h2_16, h2_8

            h32, h16, h8 = mha_ff_layer(0, hT32, hT16, None, hTf0)
            h32, h16, _ = mha_ff_layer(1, h32, h16, h8, None)

            # ---------- hypergraph conv ----------
            # out = relu((M @ h) @ Wh + bh), M = D^-1 H B^-1 H^T precomputed
            # on host (Dinv folded into Mt columns). Each core computes
            # pT[:, block b] = h_loc^T-partials, then one ReduceScatter sums
            # partials across cores and leaves each core its own node block.
            h_nat = sb.tile([128, 4, 128], F16, tag="h_nat")
            for c in range(4):
                pt = ps_m.tile([128, 128], F16, tag="m")
                nc.tensor.transpose(pt[:], h16[:, c * 128:(c + 1) * 128],
                                    c_id16[:])
                nc.vector.tensor_copy(h_nat[:, c, :], pt[:])
            rs_sb = sb.tile([128, W, NL], F16, tag="rs_sb")
            for b in range(W):
                pb = ps_s.tile([128, NL], F32, tag="scr", name=f"pconv{b}")
                for c in range(4):
                    nc.tensor.matmul(pb[:], h_nat[:, c, :],
                                     m_t[:, c, b * NL:(b + 1) * NL],
                                     start=(c == 0), stop=(c == 3))
                with nc.allow_low_precision(reason="conv partial fp16"):
                    if b % 2 == 0:
                        nc.vector.tensor_copy(rs_sb[:, b, :], pb[:])
                    else:
                        nc.scalar.copy(rs_sb[:, b, :], pb[:])
                nc.sync.dma_start(rs_in[b, :, :], rs_sb[:, b, :])
            nc.gpsimd.collective_compute(
                "ReduceScatter", ALU.add, replica_groups=RG,
                ins=[rs_in[:]], outs=[rs_out[:]])
            pT_loc = sb.tile([128, NL], F16, tag="pT_loc")
            nc.sync.dma_start(pT_loc[:], rs_out[:])
            po2 = ps_m.tile([128, NL], F32, tag="m")
            nc.tensor.matmul(po2[:], w_h[:], pT_loc[:])
            res = sb.tile([128, NL], F32, tag="res")
            nc.scalar.activation(res[:], po2[:], AF.Relu, bias=c_bh[:])
            nc.sync.dma_start(out_t[:], res[:])

    nc.compile()
    return nc


_NC = None


def _get_nc():
    global _NC
    if _NC is None:
        _NC = build_program()
    return _NC


def make_in_maps(inputs):
    x = np.asarray(inputs["x"], dtype=np.float32)
    edge = np.asarray(inputs["edge"])
    gw = {k: np.asarray(inputs[k], dtype=np.float32) for k in
          ("Wq", "bq", "Wk", "bk", "Wv", "bv", "Wo", "bo", "g_ln1", "b_ln1",
           "W1", "bf1", "W2", "bf2", "g_ln2", "b_ln2", "Wh", "bh")}

    node_idx = np.asarray(edge[0], dtype=np.int64)
    he_idx = np.asarray(edge[1], dtype=np.int64)
    counts = np.zeros((N, NE), dtype=np.float32)
    np.add.at(counts, (node_idx, he_idx), 1.0)
    Bdeg = counts.sum(axis=0)
    Ddeg = counts.sum(axis=1)
    Binv = np.where(Bdeg > 0, 1.0 / np.maximum(Bdeg, 1), 0.0).astype(np.float32)
    Dinv = np.where(Ddeg > 0, 1.0 / np.maximum(Ddeg, 1), 0.0).astype(np.float32)

    # S = H B^-1 H^T (symmetric, [N, N]); conv operand per core r:
    # Mt_r[jl, n] = S[loc_r(jl), n] * Dinv[n]
    from scipy import sparse
    Hs = sparse.csr_matrix(counts)
    S = np.asarray((Hs.multiply(Binv[None, :]) @ Hs.T).todense(),
                   dtype=np.float32)
    SD = S * Dinv[None, :]

    # CW16 pack: Wkv | Wq | Wo | W1G | Wh | id16 | w2(0.5) | ones/128
    cw16 = np.zeros((128, 1283), dtype=np.float16)
    cw16[:, 0:128] = gw["Wk"].astype(np.float16)
    cw16[:, 128:256] = gw["Wv"].astype(np.float16)
    cw16[:, 258:386] = gw["Wq"].astype(np.float16)
    cw16[:, 386:514] = gw["Wo"].astype(np.float16)
    w1g = 0.5 * gw["W1"] * gw["g_ln1"][:, None]
    cw16[:, 514:770] = w1g.astype(np.float16)
    cw16[:, 770:898] = gw["Wh"].astype(np.float16)
    cw16[:, 898:1026] = np.eye(128, dtype=np.float16)
    cw16[:, 1026:1282] = np.ascontiguousarray(
        (0.5 * gw["W2"]).reshape(2, 128, 128).transpose(1, 0, 2)
    ).reshape(128, 256).astype(np.float16)
    cw16[:, 1282] = 1.0 / 128

    # CW32 pack: bkv(131) | bq | bo | bf2 | bh
    cw32 = np.zeros((128, 135), dtype=np.float32)
    cw32[:, 0] = gw["bk"]
    cw32[:, 1:65] = gw["bv"][0:64][None, :]
    cw32[:, 66:130] = gw["bv"][64:128][None, :]
    cw32[:, 65] = 1.0
    cw32[:, 130] = 1.0
    cw32[:, 131] = gw["bq"]
    cw32[:, 132] = gw["bo"]
    cw32[:, 133] = gw["bf2"]
    cw32[:, 134] = gw["bh"]

    def lnrows(g, b):
        r = np.zeros((768,), dtype=np.float16)
        r[0:128] = g
        r[128:256] = b
        r[256:768] = 1.0
        return r

    # CROW pack: ln1r | ln2r | w2sum | ln1rn | ln2rn | g1n | fb1
    crow = np.zeros((1, 2562), dtype=np.float16)
    crow[0, 0:768] = lnrows(gw["g_ln1"], gw["b_ln1"])
    crow[0, 768:1536] = lnrows(gw["g_ln2"], gw["b_ln2"])
    crow[0, 1536:1664] = (0.5 * gw["W2"].sum(axis=0)).astype(np.float16)
    crow[0, 1664:1792] = (-gw["g_ln1"]).astype(np.float16)
    crow[0, 1792:1920] = (-gw["g_ln2"]).astype(np.float16)
    crow[0, 1920:2176] = (-0.5 * (gw["W1"].T @ gw["g_ln1"])).astype(np.float16)
    crow[0, 2176:2432] = (0.5 * (gw["W1"].T @ gw["b_ln1"])
                          + 0.5 * gw["bf1"]).astype(np.float16)
    crow[0, 2432:2560] = gw["bv"].astype(np.float16)

    shared = dict(CW16=cw16, CW32=cw32, CROW=crow)

    import ml_dtypes
    F8NP = ml_dtypes.float8_e4m3
    xTf = np.ascontiguousarray(
        x.T.reshape(128, W, NL)).astype(F8NP)
    in_maps = []
    for r in range(W):
        rows = slice(r * NL, (r + 1) * NL)
        xT = np.ascontiguousarray(x[rows, :].T)
        mt = np.ascontiguousarray(
            SD[rows, :].reshape(4, 128, N).transpose(1, 0, 2).astype(np.float16))
        m = dict(shared)
        m.update(xTf8=xTf, xT32=xT.astype(np.float32), xT16=xT.astype(np.float16),
                 Mt=mt)
        in_maps.append(m)
    return in_maps


def kernel(**inputs) -> np.ndarray:
    nc = _get_nc()
    in_maps = make_in_maps(inputs)
    res = run_bass_kernel_spmd(nc, in_maps, core_ids=list(range(W)))
    out = np.empty((N, D), dtype=np.float32)
    for r in range(W):
        out[r * NL:(r + 1) * NL, :] = res.results[r]["outT"].T
    return out


if __name__ == "__main__":
    build_program()
    print("build OK")



# revision 73
# speedup vs baseline: 1.0050x; 1.0050x over previous
"""Trainium2 Bass kernel for nn_CIE_89893665505337 (gnn_message_passing).

2x [MHA(global over 4096 nodes) + FF] transformer blocks + HypergraphConv.
8-core SPMD: nodes sharded 512/core, transposed activations hT [D=128, n],
fp16 matmul operands, f32 residual stream, AllGather for K/V, xt, e.
"""
import os
import sys

for _p in ("/opt/trn_rl_repo", "/root/.axon_site/_ro/trn_rl_repo"):
    if os.path.isdir(_p) and _p not in sys.path:
        sys.path.insert(0, _p)

import numpy as np

import concourse.bacc as bacc
import concourse.bass as bass
import concourse.tile as tile
from concourse import mybir
from concourse.bass_utils import run_bass_kernel_spmd

F32 = mybir.dt.float32
F16 = mybir.dt.float16
I32 = mybir.dt.int32
I8 = mybir.dt.int8
# exp engine split (Act / DVE only: gpsimd has no PSUM port); 32 ops/layer
EXPSPLIT = "AD"
AF = mybir.ActivationFunctionType
ALU = mybir.AluOpType

W = 8            # cores
N = 4096         # nodes
D = 128          # model dim
H = 2            # heads
DH = 64          # head dim
FF = 256         # ff dim
NE = 2048        # hyperedges
NL = N // W      # 512 local nodes
EL = NE // W     # 256 local hyperedges
EPS = 1e-5
NCH = N // 128   # 32 m-chunks
RSQRT_MAGIC = 0x5F3759DF

H_SZ = 128 * NL            # fp16 elems of hT_loc [128, 512]


def build_program():
    nc = bacc.Bacc("TRN2", target_bir_lowering=False, debug=False, num_devices=W)

    def inp(name, shape, dt=F32):
        return nc.dram_tensor(name, shape, dt, kind="ExternalInput")

    F8 = mybir.dt.float8e4
    # per-core inputs
    xT32 = inp("xT32", [128, NL])
    xT16 = inp("xT16", [128, NL], F16)
    xTf8 = inp("xTf8", [128, W, NL], F8)      # full x^T, rank-blocked, fp8
    # packed weights (one DMA each):
    # CW16: Wkv 0:258 | Wq 258:386 | Wo 386:514 | W1G 514:770 | Wh 770:898
    #       | id16 898:1026 | w2 1026:1282 | ones/128 1282:1283
    CW16 = inp("CW16", [128, 1283], F16)
    # CW32: bkv 0:131 | bq | bo | bf2 | bh
    CW32 = inp("CW32", [128, 135])
    # CROW: ln1r 0:768 | ln2r 768:1536 | w2sum 1536:1664 | ln1rn 1664:1792
    #       | ln2rn 1792:1920 | g1n 1920:2176 | fb1 2176:2432
    CROW = inp("CROW", [1, 2562], F16)
    # conv operand: Mt[p, c, n] = S[loc(c*128+p), n] * Dinv[n]  (fp16)
    Mt = inp("Mt", [128, 4, N], F16)

    out_t = nc.dram_tensor("outT", [128, NL], F32, kind="ExternalOutput")

    # AG bounce tensors (internal DRAM); outputs Shared
    kv_in = nc.dram_tensor("kv_in1", [H_SZ], F8)
    kv_out = nc.dram_tensor("kv_out1", [W, H_SZ], F8, addr_space="Shared")
    # conv ReduceScatter bounce
    rs_in = nc.dram_tensor("rs_in", [W, 128, NL], F16)
    rs_out = nc.dram_tensor("rs_out", [128, NL], F16)

    RG = [list(range(W))]

    with tile.TileContext(nc) as tc:
        with (
            tc.tile_pool(name="wpool", bufs=1) as wp,      # persistent weights/consts
            tc.tile_pool(name="sb", bufs=3) as sb,         # general sbuf tiles
            tc.tile_pool(name="kv", bufs=2) as kvp,        # kT/V per layer
            tc.tile_pool(name="expp", bufs=4) as expp,     # exp tiles
            tc.tile_pool(name="hp", bufs=1) as hp,         # conv H tiles
            tc.tile_pool(name="ps_s", bufs=2, space="PSUM") as ps_s,   # scores 2 banks each
            tc.tile_pool(name="ps_o", bufs=2, space="PSUM") as ps_o,   # attn out acc
            tc.tile_pool(name="ps_m", bufs=2, space="PSUM") as ps_m,   # misc
        ):
            # ---- load constants ----
            def load(name, shape, dram, dt=F32):
                t = wp.tile(shape, dt, name=name)
                nc.sync.dma_start(t[:], dram[:])
                return t

            F8 = mybir.dt.float8e4
            # critical path first: packed weights, then layer-1 inputs
            c_w16 = wp.tile([128, 1283], F16, name="c_w16")
            nc.gpsimd.dma_start(c_w16[:], CW16[:])
            c_w32 = wp.tile([128, 135], F32, name="c_w32")
            nc.gpsimd.dma_start(c_w32[:], CW32[:])
            hT16 = sb.tile([128, NL], F16, tag="hT16")
            nc.gpsimd.dma_start(hT16[:], xT16[:])
            c_row = wp.tile([1, 2562], F16, name="c_row")
            nc.gpsimd.dma_start(c_row[:], CROW[:])
            w_kv = c_w16
            w_k = c_w16[:, 0:128]
            w_v = c_w16[:, 128:256]
            w_q = c_w16[:, 258:386]
            w_o = c_w16[:, 386:514]
            w_1g = c_w16[:, 514:770]
            w_h = c_w16[:, 770:898]
            c_id16 = c_w16[:, 898:1026]
            c_one16 = c_w16[:, 1282:1283]
            c_bk = c_w32[:, 0:1]
            c_bq = c_w32[:, 131:132]
            c_bo = c_w32[:, 132:133]
            c_bf2 = c_w32[:, 133:134]
            c_bh = c_w32[:, 134:135]

            def w2sl(cix):
                return c_w16[:, 1026 + 128 * cix:1026 + 128 * (cix + 1)]

            # layer-1 full-x load: highest DMA priority (attention-1 gate)
            hTf0 = kvp.tile([128, W, NL], F8, tag="hT_full", name="hTf0")
            nc.sync.dma_start(hTf0[:, 0:4, :], xTf8[:, 0:4, :])
            nc.sync.dma_start(hTf0[:, 4:8, :], xTf8[:, 4:8, :])

            c_ln = [c_row[0:1, 0:768], c_row[0:1, 768:1536]]
            c_w2s = c_row[0:1, 1536:1664]
            c_lnn = [c_row[0:1, 1664:1792], c_row[0:1, 1792:1920]]
            r_g1n = c_row[0:1, 1920:2176]
            r_fb1 = c_row[0:1, 2176:2432]
            r_bv = c_row[0:1, 2432:2560]
            ones_r = c_row[0:1, 256:384]
            c_magic = wp.tile([128, 4], I32)
            nc.vector.memset(c_magic[:], RSQRT_MAGIC)
            c_ones1 = wp.tile([1, 64], F16)   # lhsT for den broadcast
            nc.vector.memset(c_ones1[:], 1.0)
            # warm the Act table set before the first real Act op needs it
            warm = wp.tile([1, 16], F32, name="warm")
            nc.vector.memset(warm[:], 0.0)
            nc.scalar.activation(warm[:], warm[:], AF.Exp)

            hT32 = sb.tile([128, NL], F32, tag="hT32")
            nc.sync.dma_start(hT32[:], xT32[:])

            # conv M operand: DMAs ride layer-2's AllGather window (idle DMA)
            m_t = hp.tile([128, 4, N], F16, name="m_t")

            def prefetch_conv_h(li, gate=None):
                if li == 1:
                    if gate is not None:
                        with nc.allow_low_precision(reason="dma gate"):
                            nc.vector.tensor_copy(m_t[0:1, 0:4, 0:1], gate)
                    for c in range(4):
                        nc.sync.dma_start(m_t[:, c, :], Mt[:, c, :])

            # ---------- helpers ----------
            def layer_norm(t32, t16_out, lnc, lncn, ff=False, t8_out=None,
                           t16_fast=False):
                """t32: [128, NL] f32 pre-LN input. Returns (h_ln f32, T strips,
                t16 fp16 copy of t32). T rows: [inv | m*inv] (+ [m | sigma]
                when ff=True, for the folded FF entry). Optionally writes an
                fp16 copy of the OUTPUT into t16_out."""
                t16 = sb.tile([128, NL], F16, tag="ln_t16")
                nc.vector.tensor_copy(t16[:], t32[:])
                t2 = sb.tile([128, NL], F16, tag="ln_t2")
                with nc.allow_low_precision(reason="LN sq to fp16"):
                    nc.scalar.square(t2[:], t32[:])
                stats = ps_m.tile([128, 8], F32, tag="m")
                for s in range(4):
                    nc.tensor.matmul(stats[:, s:s + 1],
                                     t16[:, s * 128:(s + 1) * 128], c_one16[:])
                    nc.tensor.matmul(stats[:, 4 + s:5 + s],
                                     t2[:, s * 128:(s + 1) * 128], c_one16[:])
                # stats already means (ones col = 1/128)
                m = stats[:, 0:4]
                msq = sb.tile([128, 4], F32, tag="ln_msq")
                nc.scalar.square(msq[:], m)
                ve = sb.tile([128, 4], F32, tag="ln_ve")
                nc.vector.scalar_tensor_tensor(ve[:], stats[:, 4:8], EPS, msq[:],
                                               ALU.add, ALU.subtract)
                # rsqrt via bit trick + 1 Newton iter (~0.2% rel err)
                sh = sb.tile([128, 4], I32, tag="ln_sh")
                nc.vector.tensor_scalar(sh[:], ve[:].bitcast(I32), 1, None,
                                        ALU.logical_shift_right)
                P = sb.tile([128, 8], F32, tag="ln_P")
                y = P[:, 0:4]
                nc.vector.tensor_tensor(y.bitcast(I32), c_magic[:], sh[:], ALU.subtract)
                a = sb.tile([128, 4], F32, tag="ln_a")
                nc.vector.tensor_tensor(a[:], y, y, ALU.mult)
                nc.vector.scalar_tensor_tensor(a[:], a[:], -0.5, ve[:],
                                               ALU.mult, ALU.mult)
                nc.vector.scalar_tensor_tensor(y, a[:], 1.5, y,
                                               ALU.add, ALU.mult)
                nc.vector.tensor_tensor(P[:, 4:8], m, y, ALU.mult)  # m*inv
                # strips to partition 0 via M=1 matmuls against identity:
                # out[0, j] = P16[j, s]
                nq = 4 if ff else 2
                P16 = sb.tile([128, 16], F16, tag="ln_P16")
                nc.vector.tensor_copy(P16[:, 0:8], P[:])
                if ff:
                    with nc.allow_low_precision(reason="LN strips fp16"):
                        nc.scalar.copy(P16[:, 8:12], m)
                        nc.vector.tensor_tensor(P16[:, 12:16], ve[:], y, ALU.mult)
                psT = [ps_s.tile([1, NL], F32, tag="scr", name=f"psT{q}")
                       for q in range(nq)]
                for q in range(nq):
                    for s in range(4):
                        nc.tensor.matmul(psT[q][:, s * 128:(s + 1) * 128],
                                         P16[:, 4 * q + s:4 * q + s + 1],
                                         c_id16[:])
                T = sb.tile([1, 2048], F16, tag="ln_T")
                with nc.allow_low_precision(reason="LN strip fp16"):
                    nc.scalar.copy(T[0:1, 0:NL], psT[0][:])
                    nc.vector.tensor_copy(T[0:1, NL:2 * NL], psT[1][:])
                    if ff:
                        nc.scalar.copy(T[0:1, 2 * NL:3 * NL], psT[2][:])
                        nc.vector.tensor_copy(T[0:1, 3 * NL:4 * NL], psT[3][:])
                # broadcast matmuls: A = g (x) inv ; B = (-g) (x) (m*inv) + b (x) 1
                psA = ps_m.tile([128, NL], F32, tag="m")
                psB = ps_m.tile([128, NL], F32, tag="m")
                nc.tensor.matmul(psA[:], lnc[0:1, 0:128], T[0:1, 0:NL])
                nc.tensor.matmul(psB[:], lncn[0:1, :], T[0:1, NL:2 * NL],
                                 start=True, stop=False)
                nc.tensor.matmul(psB[:], lnc[0:1, 128:256], lnc[0:1, 256:768],
                                 start=False, stop=True)
                u = sb.tile([128, NL], F32, tag="ln_u")
                nc.vector.tensor_tensor(u[:], t32[:], psA[:], ALU.mult)
                h_ln = sb.tile([128, NL], F32, tag="ln_out")
                if t8_out is not None:
                    # AG gate: fp8 output written first, directly from the add
                    with nc.allow_low_precision(reason="h8 for AllGather"):
                        nc.vector.tensor_tensor(t8_out[:], u[:], psB[:], ALU.add)
                nc.vector.tensor_tensor(h_ln[:], u[:], psB[:], ALU.add)
                if t16_out is not None:
                    with nc.allow_low_precision(reason="h16 copy"):
                        if t16_fast:
                            nc.vector.tensor_copy(t16_out[:], h_ln[:])
                        else:
                            nc.gpsimd.tensor_copy(t16_out[:], h_ln[:])
                return h_ln, T, t16

            # ---------- transformer layer ----------
            def mha_ff_layer(li, hT32_in, hT16_in, hT8_in, hTf_pre):
                # full h: layer 1 gets it free from the host input (pre-loaded);
                # layer 2 AllGathers the fp8 h produced by layer 1
                if hTf_pre is not None:
                    hTf = hTf_pre
                else:
                    hTf = kvp.tile([128, W, NL], F8, tag="hT_full")
                    nc.sync.dma_start(
                        kv_in[:].rearrange("(p j) -> p j", p=128), hT8_in[:])
                    nc.gpsimd.collective_compute(
                        "AllGather", ALU.bypass, replica_groups=RG,
                        ins=[kv_in[:]], outs=[kv_out[:]])

                ps_q = ps_m.tile([128, NL], F32, tag="m")
                nc.tensor.matmul(ps_q[:], w_q[:], hT16_in[:])
                # qz: fp8 q with a zero slot; scores use DoubleRow fp8 where
                # the k-side second slot is garbage annihilated by the zeros
                qz = sb.tile([128, 2, NL], F8, tag="qz")
                nc.gpsimd.memset(qz[:, 1, :], 0.0)
                with nc.allow_low_precision(reason="fp8 attention"):
                    nc.vector.tensor_scalar(qz[:, 0, :], ps_q[:], c_bq[:],
                                            None, ALU.add)

                # unpack h_full, then recompute kT/V locally (pipelines with attn)
                if hTf_pre is None:
                    kvv = kv_out[:].rearrange("r (p j) -> p r j", p=128)
                    nc.sync.dma_start(hTf[:, 0:1, :], kvv[:, 0:1, :])
                    nc.sync.dma_start(hTf[:, 1:4, :], kvv[:, 1:4, :])
                    nc.sync.dma_start(hTf[:, 4:8, :], kvv[:, 4:8, :])
                # kT8 layout [128, 5, 128]/rank: slots 0-3 = m-chunks, slot 4 =
                # pad so scores can view [64, 2, 128] slot pairs (cc, cc+1)
                kT8 = kvp.tile([128, W, 5, 128], F8, tag="kT_full")
                # per-head v padded to M=96: cols 0:64 v, 64 ones (denom),
                # 65:96 zero (DoubleRow needs M % 32 == 0, contiguous pairs,
                # dst partition base 0)
                vh = [kvp.tile([128, NCH, 96], F8, tag=f"v_h{h}",
                               name=f"vh{li}_{h}")
                      for h in range(H)]
                for h in range(H):
                    nc.gpsimd.memset(vh[h][:, :, 65:96], 0.0)
                    nc.gpsimd.memset(vh[h][:, :, 64:65], 1.0)
                # pad slot must be finite (NaN * 0 would poison DoubleRow)
                nc.gpsimd.memset(kT8[:, :, 4, :], 0.0)
                with nc.allow_low_precision(reason="fp8 attention"):
                    for r in range(W):
                        ps_k = ps_m.tile([128, 4, 128], F32, tag="m")
                        nc.tensor.matmul(ps_k[:], w_k[:], hTf[:, r, :])
                        if r % 2 == 0:
                            nc.scalar.activation(kT8[:, r, 0:4, :], ps_k[:],
                                                 AF.Identity, bias=c_bk[:])
                        else:
                            nc.vector.tensor_scalar(kT8[:, r, 0:4, :], ps_k[:],
                                                    c_bk[:], None, ALU.add)
                        for cp in range(2):
                            pv = ps_m.tile([128, 2, 128], F32, tag="m")
                            for j in range(2):
                                cc = 2 * cp + j
                                # bias prefilled via PE so the drain is a copy
                                nc.tensor.matmul(pv[:, j, :], ones_r[:],
                                                 r_bv[:], start=True, stop=False)
                                nc.tensor.matmul(
                                    pv[:, j, :],
                                    hTf[:, r, cc * 128:(cc + 1) * 128], w_v[:],
                                    start=False, stop=True)
                            c = 4 * r + 2 * cp
                            nc.vector.tensor_copy(vh[0][:, c:c + 2, 0:64],
                                                  pv[:, :, 0:64])
                            nc.scalar.copy(vh[1][:, c:c + 2, 0:64],
                                           pv[:, :, 64:128])

                prefetch_conv_h(li, gate=kT8[0:1, 0:1, 0:4, 0:1])

                # attention: fp8 DoubleRow scores + AV; exp split Act/DVE/Pool
                DR = mybir.MatmulPerfMode.DoubleRow
                S1, S2 = 1.4426950408889634, 54.76  # fp8e4 Schraudolph exp
                oT = sb.tile([128, NL], F16, tag="oT")
                po = [ps_o.tile([96, NL], F32, tag="o_acc", name=f"po{li}_{h}")
                      for h in range(H)]
                expi = 0
                for g in range(NCH // 2):
                    # ex_g[:, h, j, :]: head-major so AV reads a contiguous
                    # [128, 2, 512] pair per head; one exp op per chunk j
                    # covers both heads (strided output)
                    ex_g = expp.tile([128, H, 2, NL], I8, tag="exp",
                                     name=f"ex{li}_{g}")
                    for j in range(2):
                        c = 2 * g + j
                        r, cc = c // 4, c % 4
                        psc = ps_s.tile([128, H, NL], F32, tag="scr",
                                        name=f"scr{li}_{c}")
                        for h in range(H):
                            hs = slice(h * 64, (h + 1) * 64)
                            nc.tensor.matmul(
                                psc[:, h, :],
                                kT8[hs, r, cc:cc + 2, :],
                                qz[hs, :, :], perf_mode=DR)
                        eng = EXPSPLIT[expi % len(EXPSPLIT)]
                        expi += 1
                        with nc.allow_low_precision(reason="fp8 exp"):
                            if eng == "A":
                                nc.scalar.activation(
                                    ex_g[:, :, j, :].bitcast(F8), psc[:],
                                    AF.Exp, scale=0.125)
                            else:
                                nc.vector.tensor_scalar(ex_g[:, :, j, :],
                                                        psc[:], S1, S2,
                                                        ALU.mult, ALU.add)
                    for h in range(H):
                        exr = ex_g[:, h, :, :].bitcast(F8)
                        nc.tensor.matmul(
                            po[h][:], vh[h][:, 2 * g:2 * g + 2, :], exr,
                            start=(g == 0), stop=(g == NCH // 2 - 1),
                            perf_mode=DR)
                for h in range(H):
                    hs = slice(h * 64, (h + 1) * 64)
                    # normalize: fast recip of den row, broadcast, multiply
                    den32 = sb.tile([1, NL], F32, tag="den32")
                    nc.scalar.copy(den32[:], po[h][64:65, :])
                    rden32 = sb.tile([1, NL], F32, tag="rden32")
                    nc.vector.reciprocal_approx_fast(rden32[:], den32[:])
                    rden = sb.tile([1, NL], F16, tag="rden")
                    with nc.allow_low_precision(reason="attn denom recip fp16"):
                        nc.scalar.copy(rden[:], rden32[:])
                    pden = ps_m.tile([64, NL], F32, tag="m")
                    nc.tensor.matmul(pden[:], c_ones1[:], rden[:])
                    denB = sb.tile([64, NL], F16, tag="denB")
                    with nc.allow_low_precision(reason="denB fp16"):
                        nc.scalar.copy(denB[:], pden[:])
                    nc.vector.tensor_tensor(oT[hs, :], po[h][0:64, :], denB[:],
                                            ALU.mult)

                # o-projection + residual
                ps_p = ps_m.tile([128, NL], F32, tag="m")
                nc.tensor.matmul(ps_p[:], w_o[:], oT[:])
                t1 = sb.tile([128, NL], F32, tag="resid1")
                nc.vector.scalar_tensor_tensor(t1[:], ps_p[:], c_bo[:], hT32_in[:],
                                               ALU.add, ALU.add)

                h1_32, T1, t1_16 = layer_norm(t1, None, c_ln[0], c_lnn[0],
                                              ff=True)

                # FF folded through LN1: tanh arg = inv (.) [W1G^T t1
                # + G1N (x) m + FB1 (x) sigma]; sigmoid affine folded into
                # host-scaled W2 (0.5*W2) plus const row 0.5*colsum(W2)
                psI = ps_s.tile([128, NL], F32, tag="scr", name="psI")
                nc.tensor.matmul(psI[:], c_ln[0][0:1, 256:384], T1[0:1, 0:NL])
                invB = sb.tile([128, NL], F16, tag="invB")
                with nc.allow_low_precision(reason="invB fp16"):
                    nc.scalar.copy(invB[:], psI[:])
                z = sb.tile([128, 2, NL], F16, tag="z")
                zarg = sb.tile([128, 2, NL], F16, tag="zarg")
                for f in range(2):
                    pz = ps_m.tile([128, NL], F32, tag="m")
                    nc.tensor.matmul(pz[:], w_1g[:, f * 128:(f + 1) * 128],
                                     t1_16[:], start=True, stop=False)
                    nc.tensor.matmul(pz[:], r_g1n[0:1, f * 128:(f + 1) * 128],
                                     T1[0:1, 2 * NL:3 * NL],
                                     start=False, stop=False)
                    nc.tensor.matmul(pz[:], r_fb1[0:1, f * 128:(f + 1) * 128],
                                     T1[0:1, 3 * NL:4 * NL],
                                     start=False, stop=True)
                    with nc.allow_low_precision(reason="tanh arg fp16"):
                        nc.vector.tensor_tensor(zarg[:, f, :], pz[:], invB[:],
                                                ALU.mult)
                ps_f = ps_m.tile([128, NL], F32, tag="m")
                nc.tensor.matmul(ps_f[:], c_w2s[:], c_ln[0][0:1, 256:768],
                                 start=True, stop=False)
                for f in range(2):
                    nc.scalar.activation(z[:, f, :], zarg[:, f, :], AF.Tanh)
                    nc.tensor.matmul(ps_f[:], w2sl(f), z[:, f, :],
                                     start=False, stop=(f == 1))
                t2 = sb.tile([128, NL], F32, tag="resid2")
                nc.vector.scalar_tensor_tensor(t2[:], ps_f[:], c_bf2[:], h1_32[:],
                                               ALU.add, ALU.add)

                h2_16 = sb.tile([128, NL], F16, tag="hT16")
                h2_8 = None
                if li == 0:
                    h2_8 = sb.tile([128, NL], F8, tag="hT8")
                h2_32, _, _ = layer_norm(t2, h2_16, c_ln[1], c_lnn[1],
                                         t8_out=h2_8, t16_fast=(li == 1))
                return h2_32, h2_16, h2_8

            h32, h16, h8 = mha_ff_layer(0, hT32, hT16, None, hTf0)
            h32, h16, _ = mha_ff_layer(1, h32, h16, h8, None)

            # ---------- hypergraph conv ----------
            # out = relu((M @ h) @ Wh + bh), M = D^-1 H B^-1 H^T precomputed
            # on host (Dinv folded into Mt columns). Each core computes
            # pT[:, block b] = h_loc^T-partials, then one ReduceScatter sums
            # partials across cores and leaves each core its own node block.
            h_nat = sb.tile([128, 4, 128], F16, tag="h_nat")
            for c in range(4):
                pt = ps_m.tile([128, 128], F16, tag="m")
                nc.tensor.transpose(pt[:], h16[:, c * 128:(c + 1) * 128],
                                    c_id16[:])
                nc.vector.tensor_copy(h_nat[:, c, :], pt[:])
            rs_sb = sb.tile([128, W, NL], F16, tag="rs_sb")
            for b in range(W):
                pb = ps_s.tile([128, NL], F32, tag="scr", name=f"pconv{b}")
                for c in range(4):
                    nc.tensor.matmul(pb[:], h_nat[:, c, :],
                                     m_t[:, c, b * NL:(b + 1) * NL],
                                     start=(c == 0), stop=(c == 3))
                with nc.allow_low_precision(reason="conv partial fp16"):
                    if b % 2 == 0:
                        nc.vector.tensor_copy(rs_sb[:, b, :], pb[:])
                    else:
                        nc.scalar.copy(rs_sb[:, b, :], pb[:])
                nc.sync.dma_start(rs_in[b, :, :], rs_sb[:, b, :])
            nc.gpsimd.collective_compute(
                "ReduceScatter", ALU.add, replica_groups=RG,
                ins=[rs_in[:]], outs=[rs_out[:]])
            pT_loc = sb.tile([128, NL], F16, tag="pT_loc")
            nc.sync.dma_start(pT_loc[:], rs_out[:])
            po2 = ps_m.tile([128, NL], F32, tag="m")
            nc.tensor.matmul(po2[:], w_h[:], pT_loc[:])
            res = sb.tile([128, NL], F32, tag="res")
            nc.scalar.activation(res[:], po2[:], AF.Relu, bias=c_bh[:])
            nc.sync.dma_start(out_t[:], res[:])

    nc.compile()
    return nc


_NC = None


def _get_nc():
    global _NC
    if _NC is None:
        _NC = build_program()
    return _NC


def make_in_maps(inputs):
    x = np.asarray(inputs["x"], dtype=np.float32)
    edge = np.asarray(inputs["edge"])
    gw = {k: np.asarray(inputs[k], dtype=np.float32) for k in
          ("Wq", "bq", "Wk", "bk", "Wv", "bv", "Wo", "bo", "g_ln1", "b_ln1",
           "W1", "bf1", "W2", "bf2", "g_ln2", "b_ln2", "Wh", "bh")}

    node_idx = np.asarray(edge[0], dtype=np.int64)
    he_idx = np.asarray(edge[1], dtype=np.int64)
    counts = np.zeros((N, NE), dtype=np.float32)
    np.add.at(counts, (node_idx, he_idx), 1.0)
    Bdeg = counts.sum(axis=0)
    Ddeg = counts.sum(axis=1)
    Binv = np.where(Bdeg > 0, 1.0 / np.maximum(Bdeg, 1), 0.0).astype(np.float32)
    Dinv = np.where(Ddeg > 0, 1.0 / np.maximum(Ddeg, 1), 0.0).astype(np.float32)

    # S = H B^-1 H^T (symmetric, [N, N]); conv operand per core r:
    # Mt_r[jl, n] = S[loc_r(jl), n] * Dinv[n]
    from scipy import sparse
    Hs = sparse.csr_matrix(counts)
    S = np.asarray((Hs.multiply(Binv[None, :]) @ Hs.T).todense(),
                   dtype=np.float32)
    SD = S * Dinv[None, :]

    # CW16 pack: Wkv | Wq | Wo | W1G | Wh | id16 | w2(0.5) | ones/128
    cw16 = np.zeros((128, 1283), dtype=np.float16)
    cw16[:, 0:128] = gw["Wk"].astype(np.float16)
    cw16[:, 128:256] = gw["Wv"].astype(np.float16)
    cw16[:, 258:386] = gw["Wq"].astype(np.float16)
    cw16[:, 386:514] = gw["Wo"].astype(np.float16)
    w1g = 0.5 * gw["W1"] * gw["g_ln1"][:, None]
    cw16[:, 514:770] = w1g.astype(np.float16)
    cw16[:, 770:898] = gw["Wh"].astype(np.float16)
    cw16[:, 898:1026] = np.eye(128, dtype=np.float16)
    cw16[:, 1026:1282] = np.ascontiguousarray(
        (0.5 * gw["W2"]).reshape(2, 128, 128).transpose(1, 0, 2)
    ).reshape(128, 256).astype(np.float16)
    cw16[:, 1282] = 1.0 / 128

    # CW32 pack: bkv(131) | bq | bo | bf2 | bh
    cw32 = np.zeros((128, 135), dtype=np.float32)
    cw32[:, 0] = gw["bk"]
    cw32[:, 1:65] = gw["bv"][0:64][None, :]
    cw32[:, 66:130] = gw["bv"][64:128][None, :]
    cw32[:, 65] = 1.0
    cw32[:, 130] = 1.0
    cw32[:, 131] = gw["bq"]
    cw32[:, 132] = gw["bo"]
    cw32[:, 133] = gw["bf2"]
    cw32[:, 134] = gw["bh"]

    def lnrows(g, b):
        r = np.zeros((768,), dtype=np.float16)
        r[0:128] = g
        r[128:256] = b
        r[256:768] = 1.0
        return r

    # CROW pack: ln1r | ln2r | w2sum | ln1rn | ln2rn | g1n | fb1
    crow = np.zeros((1, 2562), dtype=np.float16)
    crow[0, 0:768] = lnrows(gw["g_ln1"], gw["b_ln1"])
    crow[0, 768:1536] = lnrows(gw["g_ln2"], gw["b_ln2"])
    crow[0, 1536:1664] = (0.5 * gw["W2"].sum(axis=0)).astype(np.float16)
    crow[0, 1664:1792] = (-gw["g_ln1"]).astype(np.float16)
    crow[0, 1792:1920] = (-gw["g_ln2"]).astype(np.float16)
    crow[0, 1920:2176] = (-0.5 * (gw["W1"].T @ gw["g_ln1"])).astype(np.float16)
    crow[0, 2176:2432] = (0.5 * (gw["W1"].T @ gw["b_ln1"])
                          + 0.5 * gw["bf1"]).astype(np.float16)
    crow[0, 2432:2560] = gw["bv"].astype(np.float16)

    shared = dict(CW16=cw16, CW32=cw32, CROW=crow)

    import ml_dtypes
    F8NP = ml_dtypes.float8_e4m3
    xTf = np.ascontiguousarray(
        x.T.reshape(128, W, NL)).astype(F8NP)
    in_maps = []
    for r in range(W):
        rows = slice(r * NL, (r + 1) * NL)
        xT = np.ascontiguousarray(x[rows, :].T)
        mt = np.ascontiguousarray(
            SD[rows, :].reshape(4, 128, N).transpose(1, 0, 2).astype(np.float16))
        m = dict(shared)
        m.update(xTf8=xTf, xT32=xT.astype(np.float32), xT16=xT.astype(np.float16),
                 Mt=mt)
        in_maps.append(m)
    return in_maps


def kernel(**inputs) -> np.ndarray:
    nc = _get_nc()
    in_maps = make_in_maps(inputs)
    res = run_bass_kernel_spmd(nc, in_maps, core_ids=list(range(W)))
    out = np.empty((N, D), dtype=np.float32)
    for r in range(W):
        out[r * NL:(r + 1) * NL, :] = res.results[r]["outT"].T
    return out


if __name__ == "__main__":
    build_program()
    print("build OK")



# revision 78
# speedup vs baseline: 1.0051x; 1.0001x over previous
"""Trainium2 Bass kernel for nn_CIE_89893665505337 (gnn_message_passing).

2x [MHA(global over 4096 nodes) + FF] transformer blocks + HypergraphConv.
8-core SPMD: nodes sharded 512/core, transposed activations hT [D=128, n],
fp16 matmul operands, f32 residual stream, AllGather for K/V, xt, e.
"""
import os
import sys

for _p in ("/opt/trn_rl_repo", "/root/.axon_site/_ro/trn_rl_repo"):
    if os.path.isdir(_p) and _p not in sys.path:
        sys.path.insert(0, _p)

import numpy as np

import concourse.bacc as bacc
import concourse.bass as bass
import concourse.tile as tile
from concourse import mybir
from concourse.bass_utils import run_bass_kernel_spmd

F32 = mybir.dt.float32
F16 = mybir.dt.float16
I32 = mybir.dt.int32
I8 = mybir.dt.int8
# exp engine split (Act / DVE only: gpsimd has no PSUM port); 32 ops/layer
EXPSPLIT = "AD"
AF = mybir.ActivationFunctionType
ALU = mybir.AluOpType

W = 8            # cores
N = 4096         # nodes
D = 128          # model dim
H = 2            # heads
DH = 64          # head dim
FF = 256         # ff dim
NE = 2048        # hyperedges
NL = N // W      # 512 local nodes
EL = NE // W     # 256 local hyperedges
EPS = 1e-5
NCH = N // 128   # 32 m-chunks
RSQRT_MAGIC = 0x5F3759DF

H_SZ = 128 * NL            # fp16 elems of hT_loc [128, 512]


def build_program():
    nc = bacc.Bacc("TRN2", target_bir_lowering=False, debug=False, num_devices=W)

    def inp(name, shape, dt=F32):
        return nc.dram_tensor(name, shape, dt, kind="ExternalInput")

    F8 = mybir.dt.float8e4
    # per-core inputs
    xT32 = inp("xT32", [128, NL])
    xT16 = inp("xT16", [128, NL], F16)
    xTf8 = inp("xTf8", [128, W, NL], F8)      # full x^T, rank-blocked, fp8
    # packed weights (one DMA each):
    # CW16: Wkv 0:258 | Wq 258:386 | Wo 386:514 | W1G 514:770 | Wh 770:898
    #       | id16 898:1026 | w2 1026:1282 | ones/128 1282:1283
    CW16 = inp("CW16", [128, 1283], F16)
    # CW32: bkv 0:131 | bq | bo | bf2 | bh
    CW32 = inp("CW32", [128, 135])
    # CROW: ln1r 0:768 | ln2r 768:1536 | w2sum 1536:1664 | ln1rn 1664:1792
    #       | ln2rn 1792:1920 | g1n 1920:2176 | fb1 2176:2432
    CROW = inp("CROW", [1, 2562], F16)
    # conv operand: Mt[p, c, n] = S[loc(c*128+p), n] * Dinv[n]  (fp16)
    Mt = inp("Mt", [128, 4, N], F16)

    out_t = nc.dram_tensor("outT", [128, NL], F32, kind="ExternalOutput")

    # AG bounce tensors (internal DRAM); outputs Shared
    kv_in = nc.dram_tensor("kv_in1", [H_SZ], F8)
    kv_out = nc.dram_tensor("kv_out1", [W, H_SZ], F8, addr_space="Shared")
    # conv ReduceScatter bounce
    rs_in = nc.dram_tensor("rs_in", [W, 128, NL], F16)
    rs_out = nc.dram_tensor("rs_out", [128, NL], F16)

    RG = [list(range(W))]

    with tile.TileContext(nc) as tc:
        with (
            tc.tile_pool(name="wpool", bufs=1) as wp,      # persistent weights/consts
            tc.tile_pool(name="sb", bufs=3) as sb,         # general sbuf tiles
            tc.tile_pool(name="kv", bufs=2) as kvp,        # kT/V per layer
            tc.tile_pool(name="expp", bufs=4) as expp,     # exp tiles
            tc.tile_pool(name="hp", bufs=1) as hp,         # conv H tiles
            tc.tile_pool(name="ps_s", bufs=2, space="PSUM") as ps_s,   # scores 2 banks each
            tc.tile_pool(name="ps_o", bufs=2, space="PSUM") as ps_o,   # attn out acc
            tc.tile_pool(name="ps_m", bufs=2, space="PSUM") as ps_m,   # misc
        ):
            # ---- load constants ----
            def load(name, shape, dram, dt=F32):
                t = wp.tile(shape, dt, name=name)
                nc.sync.dma_start(t[:], dram[:])
                return t

            F8 = mybir.dt.float8e4
            # critical path first: packed weights, then layer-1 inputs
            c_w16 = wp.tile([128, 1283], F16, name="c_w16")
            nc.gpsimd.dma_start(c_w16[:], CW16[:])
            c_w32 = wp.tile([128, 135], F32, name="c_w32")
            nc.gpsimd.dma_start(c_w32[:], CW32[:])
            hT16 = sb.tile([128, NL], F16, tag="hT16")
            nc.gpsimd.dma_start(hT16[:], xT16[:])
            c_row = wp.tile([1, 2562], F16, name="c_row")
            nc.gpsimd.dma_start(c_row[:], CROW[:])
            w_kv = c_w16
            w_k = c_w16[:, 0:128]
            w_v = c_w16[:, 128:256]
            w_q = c_w16[:, 258:386]
            w_o = c_w16[:, 386:514]
            w_1g = c_w16[:, 514:770]
            w_h = c_w16[:, 770:898]
            c_id16 = c_w16[:, 898:1026]
            c_one16 = c_w16[:, 1282:1283]
            c_bk = c_w32[:, 0:1]
            c_bq = c_w32[:, 131:132]
            c_bo = c_w32[:, 132:133]
            c_bf2 = c_w32[:, 133:134]
            c_bh = c_w32[:, 134:135]

            def w2sl(cix):
                return c_w16[:, 1026 + 128 * cix:1026 + 128 * (cix + 1)]

            # layer-1 full-x load: highest DMA priority (attention-1 gate)
            hTf0 = kvp.tile([128, W, NL], F8, tag="hT_full", name="hTf0")
            nc.sync.dma_start(hTf0[:, 0:4, :], xTf8[:, 0:4, :])
            nc.sync.dma_start(hTf0[:, 4:8, :], xTf8[:, 4:8, :])

            c_ln = [c_row[0:1, 0:768], c_row[0:1, 768:1536]]
            c_w2s = c_row[0:1, 1536:1664]
            c_lnn = [c_row[0:1, 1664:1792], c_row[0:1, 1792:1920]]
            r_g1n = c_row[0:1, 1920:2176]
            r_fb1 = c_row[0:1, 2176:2432]
            r_bv = c_row[0:1, 2432:2560]
            ones_r = c_row[0:1, 256:384]
            c_magic = wp.tile([128, 4], I32)
            nc.vector.memset(c_magic[:], RSQRT_MAGIC)
            c_ones1 = wp.tile([1, 64], F16)   # lhsT for den broadcast
            nc.vector.memset(c_ones1[:], 1.0)
            # warm the Act table set before the first real Act op needs it
            warm = wp.tile([1, 16], F32, name="warm")
            nc.vector.memset(warm[:], 0.0)
            nc.scalar.activation(warm[:], warm[:], AF.Exp)

            hT32 = sb.tile([128, NL], F32, tag="hT32")
            nc.sync.dma_start(hT32[:], xT32[:])

            # conv M operand: DMAs ride layer-2's AllGather window (idle DMA)
            m_t = hp.tile([128, 4, N], F16, name="m_t")

            def prefetch_conv_h(li, gate=None):
                if li == 1:
                    if gate is not None:
                        with nc.allow_low_precision(reason="dma gate"):
                            nc.vector.tensor_copy(m_t[0:1, 0:4, 0:1], gate)
                    for c in range(4):
                        nc.sync.dma_start(m_t[:, c, :], Mt[:, c, :])

            # ---------- helpers ----------
            def layer_norm(t32, t16_out, lnc, lncn, ff=False, t8_out=None,
                           t16_fast=False):
                """t32: [128, NL] f32 pre-LN input. Returns (h_ln f32, T strips,
                t16 fp16 copy of t32). T rows: [inv | m*inv] (+ [m | sigma]
                when ff=True, for the folded FF entry). Optionally writes an
                fp16 copy of the OUTPUT into t16_out."""
                t16 = sb.tile([128, NL], F16, tag="ln_t16")
                nc.vector.tensor_copy(t16[:], t32[:])
                t2 = sb.tile([128, NL], F16, tag="ln_t2")
                with nc.allow_low_precision(reason="LN sq to fp16"):
                    nc.scalar.square(t2[:], t32[:])
                stats = ps_m.tile([128, 8], F32, tag="m")
                for s in range(4):
                    nc.tensor.matmul(stats[:, s:s + 1],
                                     t16[:, s * 128:(s + 1) * 128], c_one16[:])
                    nc.tensor.matmul(stats[:, 4 + s:5 + s],
                                     t2[:, s * 128:(s + 1) * 128], c_one16[:])
                # stats already means (ones col = 1/128)
                m = stats[:, 0:4]
                msq = sb.tile([128, 4], F32, tag="ln_msq")
                nc.scalar.square(msq[:], m)
                ve = sb.tile([128, 4], F32, tag="ln_ve")
                nc.vector.scalar_tensor_tensor(ve[:], stats[:, 4:8], EPS, msq[:],
                                               ALU.add, ALU.subtract)
                # rsqrt via bit trick + 1 Newton iter (~0.2% rel err)
                sh = sb.tile([128, 4], I32, tag="ln_sh")
                nc.vector.tensor_scalar(sh[:], ve[:].bitcast(I32), 1, None,
                                        ALU.logical_shift_right)
                P = sb.tile([128, 8], F32, tag="ln_P")
                y = P[:, 0:4]
                nc.vector.tensor_tensor(y.bitcast(I32), c_magic[:], sh[:], ALU.subtract)
                a = sb.tile([128, 4], F32, tag="ln_a")
                nc.vector.tensor_tensor(a[:], y, y, ALU.mult)
                nc.vector.scalar_tensor_tensor(a[:], a[:], -0.5, ve[:],
                                               ALU.mult, ALU.mult)
                nc.vector.scalar_tensor_tensor(y, a[:], 1.5, y,
                                               ALU.add, ALU.mult)
                nc.vector.tensor_tensor(P[:, 4:8], m, y, ALU.mult)  # m*inv
                # strips to partition 0 via M=1 matmuls against identity:
                # out[0, j] = P16[j, s]
                nq = 4 if ff else 2
                P16 = sb.tile([128, 16], F16, tag="ln_P16")
                nc.vector.tensor_copy(P16[:, 0:8], P[:])
                if ff:
                    with nc.allow_low_precision(reason="LN strips fp16"):
                        nc.scalar.copy(P16[:, 8:12], m)
                        nc.vector.tensor_tensor(P16[:, 12:16], ve[:], y, ALU.mult)
                psT = [ps_s.tile([1, NL], F32, tag="scr", name=f"psT{q}")
                       for q in range(nq)]
                for q in range(nq):
                    for s in range(4):
                        nc.tensor.matmul(psT[q][:, s * 128:(s + 1) * 128],
                                         P16[:, 4 * q + s:4 * q + s + 1],
                                         c_id16[:])
                T = sb.tile([1, 2048], F16, tag="ln_T")
                with nc.allow_low_precision(reason="LN strip fp16"):
                    nc.scalar.copy(T[0:1, 0:NL], psT[0][:])
                    nc.vector.tensor_copy(T[0:1, NL:2 * NL], psT[1][:])
                    if ff:
                        nc.scalar.copy(T[0:1, 2 * NL:3 * NL], psT[2][:])
                        nc.vector.tensor_copy(T[0:1, 3 * NL:4 * NL], psT[3][:])
                # broadcast matmuls: A = g (x) inv ; B = (-g) (x) (m*inv) + b (x) 1
                psA = ps_m.tile([128, NL], F32, tag="m")
                psB = ps_m.tile([128, NL], F32, tag="m")
                nc.tensor.matmul(psA[:], lnc[0:1, 0:128], T[0:1, 0:NL])
                nc.tensor.matmul(psB[:], lncn[0:1, :], T[0:1, NL:2 * NL],
                                 start=True, stop=False)
                nc.tensor.matmul(psB[:], lnc[0:1, 128:256], lnc[0:1, 256:768],
                                 start=False, stop=True)
                u = sb.tile([128, NL], F32, tag="ln_u")
                nc.vector.tensor_tensor(u[:], t32[:], psA[:], ALU.mult)
                h_ln = sb.tile([128, NL], F32, tag="ln_out")
                if t8_out is not None:
                    # AG gate: fp8 output written first, directly from the add
                    with nc.allow_low_precision(reason="h8 for AllGather"):
                        nc.vector.tensor_tensor(t8_out[:], u[:], psB[:], ALU.add)
                nc.vector.tensor_tensor(h_ln[:], u[:], psB[:], ALU.add)
                if t16_out is not None:
                    with nc.allow_low_precision(reason="h16 copy"):
                        if t16_fast:
                            nc.vector.tensor_copy(t16_out[:], h_ln[:])
                        else:
                            nc.gpsimd.tensor_copy(t16_out[:], h_ln[:])
                return h_ln, T, t16

            # ---------- transformer layer ----------
            def mha_ff_layer(li, hT32_in, hT16_in, hT8_in, hTf_pre):
                # full h: layer 1 gets it free from the host input (pre-loaded);
                # layer 2 AllGathers the fp8 h produced by layer 1
                if hTf_pre is not None:
                    hTf = hTf_pre
                else:
                    hTf = kvp.tile([128, W, NL], F8, tag="hT_full")
                    nc.sync.dma_start(
                        kv_in[:].rearrange("(p j) -> p j", p=128), hT8_in[:])
                    nc.gpsimd.collective_compute(
                        "AllGather", ALU.bypass, replica_groups=RG,
                        ins=[kv_in[:]], outs=[kv_out[:]])

                ps_q = ps_m.tile([128, NL], F32, tag="m")
                nc.tensor.matmul(ps_q[:], w_q[:], hT16_in[:])
                # qz: fp8 q with a zero slot; scores use DoubleRow fp8 where
                # the k-side second slot is garbage annihilated by the zeros
                qz = sb.tile([128, 2, NL], F8, tag="qz")
                nc.gpsimd.memset(qz[:, 1, :], 0.0)
                with nc.allow_low_precision(reason="fp8 attention"):
                    nc.vector.tensor_scalar(qz[:, 0, :], ps_q[:], c_bq[:],
                                            None, ALU.add)

                # unpack h_full, then recompute kT/V locally (pipelines with attn)
                if hTf_pre is None:
                    kvv = kv_out[:].rearrange("r (p j) -> p r j", p=128)
                    nc.sync.dma_start(hTf[:, 0:1, :], kvv[:, 0:1, :])
                    nc.sync.dma_start(hTf[:, 1:4, :], kvv[:, 1:4, :])
                    nc.sync.dma_start(hTf[:, 4:8, :], kvv[:, 4:8, :])
                # kT8 layout [128, 5, 128]/rank: slots 0-3 = m-chunks, slot 4 =
                # pad so scores can view [64, 2, 128] slot pairs (cc, cc+1)
                kT8 = kvp.tile([128, W, 5, 128], F8, tag="kT_full")
                # per-head v padded to M=96: cols 0:64 v, 64 ones (denom),
                # 65:96 zero (DoubleRow needs M % 32 == 0, contiguous pairs,
                # dst partition base 0)
                vh = [kvp.tile([128, NCH, 96], F8, tag=f"v_h{h}",
                               name=f"vh{li}_{h}")
                      for h in range(H)]
                for h in range(H):
                    nc.gpsimd.memset(vh[h][:, :, 65:96], 0.0)
                    nc.gpsimd.memset(vh[h][:, :, 64:65], 1.0)
                # pad slot must be finite (NaN * 0 would poison DoubleRow)
                nc.gpsimd.memset(kT8[:, :, 4, :], 0.0)
                with nc.allow_low_precision(reason="fp8 attention"):
                    for r in range(W):
                        ps_k = ps_m.tile([128, 4, 128], F32, tag="m")
                        nc.tensor.matmul(ps_k[:], w_k[:], hTf[:, r, :])
                        if r % 2 == 0:
                            nc.scalar.activation(kT8[:, r, 0:4, :], ps_k[:],
                                                 AF.Identity, bias=c_bk[:])
                        else:
                            nc.vector.tensor_scalar(kT8[:, r, 0:4, :], ps_k[:],
                                                    c_bk[:], None, ALU.add)
                        for cp in range(2):
                            pv = ps_m.tile([128, 2, 128], F32, tag="m")
                            for j in range(2):
                                cc = 2 * cp + j
                                # bias prefilled via PE so the drain is a copy
                                nc.tensor.matmul(pv[:, j, :], ones_r[:],
                                                 r_bv[:], start=True, stop=False)
                                nc.tensor.matmul(
                                    pv[:, j, :],
                                    hTf[:, r, cc * 128:(cc + 1) * 128], w_v[:],
                                    start=False, stop=True)
                            c = 4 * r + 2 * cp
                            nc.vector.tensor_copy(vh[0][:, c:c + 2, 0:64],
                                                  pv[:, :, 0:64])
                            nc.scalar.copy(vh[1][:, c:c + 2, 0:64],
                                           pv[:, :, 64:128])

                prefetch_conv_h(li, gate=kT8[0:1, 0:1, 0:4, 0:1])

                # attention: fp8 DoubleRow scores + AV; exp split Act/DVE/Pool
                DR = mybir.MatmulPerfMode.DoubleRow
                S1, S2 = 1.4426950408889634, 54.76  # fp8e4 Schraudolph exp
                oT = sb.tile([128, NL], F16, tag="oT")
                po = [ps_o.tile([96, NL], F32, tag="o_acc", name=f"po{li}_{h}")
                      for h in range(H)]
                expi = 0
                for g in range(NCH // 2):
                    # ex_g[:, h, j, :]: head-major so AV reads a contiguous
                    # [128, 2, 512] pair per head; one exp op per chunk j
                    # covers both heads (strided output)
                    ex_g = expp.tile([128, H, 2, NL], I8, tag="exp",
                                     name=f"ex{li}_{g}")
                    for j in range(2):
                        c = 2 * g + j
                        r, cc = c // 4, c % 4
                        psc = ps_s.tile([128, H, NL], F32, tag="scr",
                                        name=f"scr{li}_{c}")
                        for h in range(H):
                            hs = slice(h * 64, (h + 1) * 64)
                            nc.tensor.matmul(
                                psc[:, h, :],
                                kT8[hs, r, cc:cc + 2, :],
                                qz[hs, :, :], perf_mode=DR)
                        eng = EXPSPLIT[expi % len(EXPSPLIT)]
                        expi += 1
                        with nc.allow_low_precision(reason="fp8 exp"):
                            if eng == "A":
                                nc.scalar.activation(
                                    ex_g[:, :, j, :].bitcast(F8), psc[:],
                                    AF.Exp, scale=0.125)
                            else:
                                nc.vector.tensor_scalar(ex_g[:, :, j, :],
                                                        psc[:], S1, S2,
                                                        ALU.mult, ALU.add)
                    for h in range(H):
                        exr = ex_g[:, h, :, :].bitcast(F8)
                        nc.tensor.matmul(
                            po[h][:], vh[h][:, 2 * g:2 * g + 2, :], exr,
                            start=(g == 0), stop=(g == NCH // 2 - 1),
                            perf_mode=DR)
                for h in range(H):
                    hs = slice(h * 64, (h + 1) * 64)
                    # normalize: fast recip of den row, broadcast, multiply
                    den32 = sb.tile([1, NL], F32, tag="den32")
                    if h == 0:
                        nc.vector.tensor_copy(den32[:], po[h][64:65, :])
                    else:
                        nc.scalar.copy(den32[:], po[h][64:65, :])
                    rden32 = sb.tile([1, NL], F32, tag="rden32")
                    nc.vector.reciprocal_approx_fast(rden32[:], den32[:])
                    rden = sb.tile([1, NL], F16, tag="rden")
                    with nc.allow_low_precision(reason="attn denom recip fp16"):
                        nc.scalar.copy(rden[:], rden32[:])
                    pden = ps_m.tile([64, NL], F32, tag="m")
                    nc.tensor.matmul(pden[:], c_ones1[:], rden[:])
                    denB = sb.tile([64, NL], F16, tag="denB")
                    with nc.allow_low_precision(reason="denB fp16"):
                        nc.scalar.copy(denB[:], pden[:])
                    nc.vector.tensor_tensor(oT[hs, :], po[h][0:64, :], denB[:],
                                            ALU.mult)

                # o-projection + residual
                ps_p = ps_m.tile([128, NL], F32, tag="m")
                nc.tensor.matmul(ps_p[:], w_o[:], oT[:])
                t1 = sb.tile([128, NL], F32, tag="resid1")
                nc.vector.scalar_tensor_tensor(t1[:], ps_p[:], c_bo[:], hT32_in[:],
                                               ALU.add, ALU.add)

                h1_32, T1, t1_16 = layer_norm(t1, None, c_ln[0], c_lnn[0],
                                              ff=True)

                # FF folded through LN1: tanh arg = inv (.) [W1G^T t1
                # + G1N (x) m + FB1 (x) sigma]; sigmoid affine folded into
                # host-scaled W2 (0.5*W2) plus const row 0.5*colsum(W2)
                psI = ps_s.tile([128, NL], F32, tag="scr", name="psI")
                nc.tensor.matmul(psI[:], c_ln[0][0:1, 256:384], T1[0:1, 0:NL])
                invB = sb.tile([128, NL], F16, tag="invB")
                with nc.allow_low_precision(reason="invB fp16"):
                    nc.scalar.copy(invB[:], psI[:])
                z = sb.tile([128, 2, NL], F16, tag="z")
                zarg = sb.tile([128, 2, NL], F16, tag="zarg")
                for f in range(2):
                    pz = ps_m.tile([128, NL], F32, tag="m")
                    nc.tensor.matmul(pz[:], w_1g[:, f * 128:(f + 1) * 128],
                                     t1_16[:], start=True, stop=False)
                    nc.tensor.matmul(pz[:], r_g1n[0:1, f * 128:(f + 1) * 128],
                                     T1[0:1, 2 * NL:3 * NL],
                                     start=False, stop=False)
                    nc.tensor.matmul(pz[:], r_fb1[0:1, f * 128:(f + 1) * 128],
                                     T1[0:1, 3 * NL:4 * NL],
                                     start=False, stop=True)
                    with nc.allow_low_precision(reason="tanh arg fp16"):
                        nc.vector.tensor_tensor(zarg[:, f, :], pz[:], invB[:],
                                                ALU.mult)
                ps_f = ps_m.tile([128, NL], F32, tag="m")
                nc.tensor.matmul(ps_f[:], c_w2s[:], c_ln[0][0:1, 256:768],
                                 start=True, stop=False)
                for f in range(2):
                    nc.scalar.activation(z[:, f, :], zarg[:, f, :], AF.Tanh)
                    nc.tensor.matmul(ps_f[:], w2sl(f), z[:, f, :],
                                     start=False, stop=(f == 1))
                t2 = sb.tile([128, NL], F32, tag="resid2")
                nc.vector.scalar_tensor_tensor(t2[:], ps_f[:], c_bf2[:], h1_32[:],
                                               ALU.add, ALU.add)

                h2_16 = sb.tile([128, NL], F16, tag="hT16")
                h2_8 = None
                if li == 0:
                    h2_8 = sb.tile([128, NL], F8, tag="hT8")
                h2_32, _, _ = layer_norm(t2, h2_16, c_ln[1], c_lnn[1],
                                         t8_out=h2_8, t16_fast=(li == 1))
                return h2_32, h2_16, h2_8

            h32, h16, h8 = mha_ff_layer(0, hT32, hT16, None, hTf0)
            h32, h16, _ = mha_ff_layer(1, h32, h16, h8, None)

            # ---------- hypergraph conv ----------
            # out = relu((M @ h) @ Wh + bh), M = D^-1 H B^-1 H^T precomputed
            # on host (Dinv folded into Mt columns). Each core computes
            # pT[:, block b] = h_loc^T-partials, then one ReduceScatter sums
            # partials across cores and leaves each core its own node block.
            h_nat = sb.tile([128, 4, 128], F16, tag="h_nat")
            for c in range(4):
                pt = ps_m.tile([128, 128], F16, tag="m")
                nc.tensor.transpose(pt[:], h16[:, c * 128:(c + 1) * 128],
                                    c_id16[:])
                nc.vector.tensor_copy(h_nat[:, c, :], pt[:])
            rs_sb = sb.tile([128, W, NL], F16, tag="rs_sb")
            for b in range(W):
                pb = ps_s.tile([128, NL], F32, tag="scr", name=f"pconv{b}")
                for c in range(4):
                    nc.tensor.matmul(pb[:], h_nat[:, c, :],
                                     m_t[:, c, b * NL:(b + 1) * NL],
                                     start=(c == 0), stop=(c == 3))
                with nc.allow_low_precision(reason="conv partial fp16"):
                    if b % 2 == 0:
                        nc.vector.tensor_copy(rs_sb[:, b, :], pb[:])
                    else:
                        nc.scalar.copy(rs_sb[:, b, :], pb[:])
                nc.sync.dma_start(rs_in[b, :, :], rs_sb[:, b, :])
            nc.gpsimd.collective_compute(
                "ReduceScatter", ALU.add, replica_groups=RG,
                ins=[rs_in[:]], outs=[rs_out[:]])
            pT_loc = sb.tile([128, NL], F16, tag="pT_loc")
            nc.sync.dma_start(pT_loc[:], rs_out[:])
            po2 = ps_m.tile([128, NL], F32, tag="m")
            nc.tensor.matmul(po2[:], w_h[:], pT_loc[:])
            res = sb.tile([128, NL], F32, tag="res")
            nc.scalar.activation(res[:], po2[:], AF.Relu, bias=c_bh[:])
            nc.sync.dma_start(out_t[:], res[:])

    nc.compile()
    return nc


_NC = None


def _get_nc():
    global _NC
    if _NC is None:
        _NC = build_program()
    return _NC


def make_in_maps(inputs):
    x = np.asarray(inputs["x"], dtype=np.float32)
    edge = np.asarray(inputs["edge"])
    gw = {k: np.asarray(inputs[k], dtype=np.float32) for k in
          ("Wq", "bq", "Wk", "bk", "Wv", "bv", "Wo", "bo", "g_ln1", "b_ln1",
           "W1", "bf1", "W2", "bf2", "g_ln2", "b_ln2", "Wh", "bh")}

    node_idx = np.asarray(edge[0], dtype=np.int64)
    he_idx = np.asarray(edge[1], dtype=np.int64)
    counts = np.zeros((N, NE), dtype=np.float32)
    np.add.at(counts, (node_idx, he_idx), 1.0)
    Bdeg = counts.sum(axis=0)
    Ddeg = counts.sum(axis=1)
    Binv = np.where(Bdeg > 0, 1.0 / np.maximum(Bdeg, 1), 0.0).astype(np.float32)
    Dinv = np.where(Ddeg > 0, 1.0 / np.maximum(Ddeg, 1), 0.0).astype(np.float32)

    # S = H B^-1 H^T (symmetric, [N, N]); conv operand per core r:
    # Mt_r[jl, n] = S[loc_r(jl), n] * Dinv[n]
    from scipy import sparse
    Hs = sparse.csr_matrix(counts)
    S = np.asarray((Hs.multiply(Binv[None, :]) @ Hs.T).todense(),
                   dtype=np.float32)
    SD = S * Dinv[None, :]

    # CW16 pack: Wkv | Wq | Wo | W1G | Wh | id16 | w2(0.5) | ones/128
    cw16 = np.zeros((128, 1283), dtype=np.float16)
    cw16[:, 0:128] = gw["Wk"].astype(np.float16)
    cw16[:, 128:256] = gw["Wv"].astype(np.float16)
    cw16[:, 258:386] = gw["Wq"].astype(np.float16)
    cw16[:, 386:514] = gw["Wo"].astype(np.float16)
    w1g = 0.5 * gw["W1"] * gw["g_ln1"][:, None]
    cw16[:, 514:770] = w1g.astype(np.float16)
    cw16[:, 770:898] = gw["Wh"].astype(np.float16)
    cw16[:, 898:1026] = np.eye(128, dtype=np.float16)
    cw16[:, 1026:1282] = np.ascontiguousarray(
        (0.5 * gw["W2"]).reshape(2, 128, 128).transpose(1, 0, 2)
    ).reshape(128, 256).astype(np.float16)
    cw16[:, 1282] = 1.0 / 128

    # CW32 pack: bkv(131) | bq | bo | bf2 | bh
    cw32 = np.zeros((128, 135), dtype=np.float32)
    cw32[:, 0] = gw["bk"]
    cw32[:, 1:65] = gw["bv"][0:64][None, :]
    cw32[:, 66:130] = gw["bv"][64:128][None, :]
    cw32[:, 65] = 1.0
    cw32[:, 130] = 1.0
    cw32[:, 131] = gw["bq"]
    cw32[:, 132] = gw["bo"]
    cw32[:, 133] = gw["bf2"]
    cw32[:, 134] = gw["bh"]

    def lnrows(g, b):
        r = np.zeros((768,), dtype=np.float16)
        r[0:128] = g
        r[128:256] = b
        r[256:768] = 1.0
        return r

    # CROW pack: ln1r | ln2r | w2sum | ln1rn | ln2rn | g1n | fb1
    crow = np.zeros((1, 2562), dtype=np.float16)
    crow[0, 0:768] = lnrows(gw["g_ln1"], gw["b_ln1"])
    crow[0, 768:1536] = lnrows(gw["g_ln2"], gw["b_ln2"])
    crow[0, 1536:1664] = (0.5 * gw["W2"].sum(axis=0)).astype(np.float16)
    crow[0, 1664:1792] = (-gw["g_ln1"]).astype(np.float16)
    crow[0, 1792:1920] = (-gw["g_ln2"]).astype(np.float16)
    crow[0, 1920:2176] = (-0.5 * (gw["W1"].T @ gw["g_ln1"])).astype(np.float16)
    crow[0, 2176:2432] = (0.5 * (gw["W1"].T @ gw["b_ln1"])
                          + 0.5 * gw["bf1"]).astype(np.float16)
    crow[0, 2432:2560] = gw["bv"].astype(np.float16)

    shared = dict(CW16=cw16, CW32=cw32, CROW=crow)

    import ml_dtypes
    F8NP = ml_dtypes.float8_e4m3
    xTf = np.ascontiguousarray(
        x.T.reshape(128, W, NL)).astype(F8NP)
    in_maps = []
    for r in range(W):
        rows = slice(r * NL, (r + 1) * NL)
        xT = np.ascontiguousarray(x[rows, :].T)
        mt = np.ascontiguousarray(
            SD[rows, :].reshape(4, 128, N).transpose(1, 0, 2).astype(np.float16))
        m = dict(shared)
        m.update(xTf8=xTf, xT32=xT.astype(np.float32), xT16=xT.astype(np.float16),
                 Mt=mt)
        in_maps.append(m)
    return in_maps


def kernel(**inputs) -> np.ndarray:
    nc = _get_nc()
    in_maps = make_in_maps(inputs)
    res = run_bass_kernel_spmd(nc, in_maps, core_ids=list(range(W)))
    out = np.empty((N, D), dtype=np.float32)
    for r in range(W):
        out[r * NL:(r + 1) * NL, :] = res.results[r]["outT"].T
    return out


if __name__ == "__main__":
    build_program()
    print("build OK")



# revision 81
# speedup vs baseline: 1.0065x; 1.0014x over previous
"""Trainium2 Bass kernel for nn_CIE_89893665505337 (gnn_message_passing).

2x [MHA(global over 4096 nodes) + FF] transformer blocks + HypergraphConv.
8-core SPMD: nodes sharded 512/core, transposed activations hT [D=128, n],
fp16 matmul operands, f32 residual stream, AllGather for K/V, xt, e.
"""
import os
import sys

for _p in ("/opt/trn_rl_repo", "/root/.axon_site/_ro/trn_rl_repo"):
    if os.path.isdir(_p) and _p not in sys.path:
        sys.path.insert(0, _p)

import numpy as np

import concourse.bacc as bacc
import concourse.bass as bass
import concourse.tile as tile
from concourse import mybir
from concourse.bass_utils import run_bass_kernel_spmd

F32 = mybir.dt.float32
F16 = mybir.dt.float16
I32 = mybir.dt.int32
I8 = mybir.dt.int8
# exp engine split (Act / DVE only: gpsimd has no PSUM port); 32 ops/layer
EXPSPLIT = "AD"
AF = mybir.ActivationFunctionType
ALU = mybir.AluOpType

W = 8            # cores
N = 4096         # nodes
D = 128          # model dim
H = 2            # heads
DH = 64          # head dim
FF = 256         # ff dim
NE = 2048        # hyperedges
NL = N // W      # 512 local nodes
EL = NE // W     # 256 local hyperedges
EPS = 1e-5
NCH = N // 128   # 32 m-chunks
RSQRT_MAGIC = 0x5F3759DF

H_SZ = 128 * NL            # fp16 elems of hT_loc [128, 512]


def build_program():
    nc = bacc.Bacc("TRN2", target_bir_lowering=False, debug=False, num_devices=W)

    def inp(name, shape, dt=F32):
        return nc.dram_tensor(name, shape, dt, kind="ExternalInput")

    F8 = mybir.dt.float8e4
    # per-core inputs
    xT32 = inp("xT32", [128, NL])
    xT16 = inp("xT16", [128, NL], F16)
    xTf8 = inp("xTf8", [128, W, NL], F8)      # full x^T, rank-blocked, fp8
    # packed weights (one DMA each):
    # CW16: Wkv 0:258 | Wq 258:386 | Wo 386:514 | W1G 514:770 | Wh 770:898
    #       | id16 898:1026 | w2 1026:1282 | ones/128 1282:1283
    CW16 = inp("CW16", [128, 1283], F16)
    # CW32: bkv 0:131 | bq | bo | bf2 | bh
    CW32 = inp("CW32", [128, 135])
    # CROW: ln1r 0:768 | ln2r 768:1536 | w2sum 1536:1664 | ln1rn 1664:1792
    #       | ln2rn 1792:1920 | g1n 1920:2176 | fb1 2176:2432
    CROW = inp("CROW", [1, 2562], F16)
    # conv operand: Mt[p, c, n] = S[loc(c*128+p), n] * Dinv[n]  (fp16)
    Mt = inp("Mt", [128, 4, N], F16)

    out_t = nc.dram_tensor("outT", [128, NL], F32, kind="ExternalOutput")

    # AG bounce tensors (internal DRAM); outputs Shared
    kv_in = nc.dram_tensor("kv_in1", [H_SZ], F8)
    kv_out = nc.dram_tensor("kv_out1", [W, H_SZ], F8, addr_space="Shared")
    # conv ReduceScatter bounce
    rs_in = nc.dram_tensor("rs_in", [W, 128, NL], F16)
    rs_out = nc.dram_tensor("rs_out", [128, NL], F16)

    RG = [list(range(W))]

    with tile.TileContext(nc) as tc:
        with (
            tc.tile_pool(name="wpool", bufs=1) as wp,      # persistent weights/consts
            tc.tile_pool(name="sb", bufs=3) as sb,         # general sbuf tiles
            tc.tile_pool(name="kv", bufs=2) as kvp,        # kT/V per layer
            tc.tile_pool(name="expp", bufs=4) as expp,     # exp tiles
            tc.tile_pool(name="hp", bufs=1) as hp,         # conv H tiles
            tc.tile_pool(name="ps_s", bufs=2, space="PSUM") as ps_s,   # scores 2 banks each
            tc.tile_pool(name="ps_o", bufs=2, space="PSUM") as ps_o,   # attn out acc
            tc.tile_pool(name="ps_m", bufs=2, space="PSUM") as ps_m,   # misc
        ):
            # ---- load constants ----
            def load(name, shape, dram, dt=F32):
                t = wp.tile(shape, dt, name=name)
                nc.sync.dma_start(t[:], dram[:])
                return t

            F8 = mybir.dt.float8e4
            # critical path first: packed weights, then layer-1 inputs
            c_w16 = wp.tile([128, 1283], F16, name="c_w16")
            nc.gpsimd.dma_start(c_w16[:], CW16[:])
            c_w32 = wp.tile([128, 135], F32, name="c_w32")
            nc.gpsimd.dma_start(c_w32[:], CW32[:])
            hT16 = sb.tile([128, NL], F16, tag="hT16")
            nc.gpsimd.dma_start(hT16[:], xT16[:])
            c_row = wp.tile([1, 2562], F16, name="c_row")
            nc.gpsimd.dma_start(c_row[:], CROW[:])
            w_kv = c_w16
            w_k = c_w16[:, 0:128]
            w_v = c_w16[:, 128:256]
            w_q = c_w16[:, 258:386]
            w_o = c_w16[:, 386:514]
            w_1g = c_w16[:, 514:770]
            w_h = c_w16[:, 770:898]
            c_id16 = c_w16[:, 898:1026]
            c_one16 = c_w16[:, 1282:1283]
            c_bk = c_w32[:, 0:1]
            c_bq = c_w32[:, 131:132]
            c_bo = c_w32[:, 132:133]
            c_bf2 = c_w32[:, 133:134]
            c_bh = c_w32[:, 134:135]

            def w2sl(cix):
                return c_w16[:, 1026 + 128 * cix:1026 + 128 * (cix + 1)]

            # layer-1 full-x load: highest DMA priority (attention-1 gate)
            hTf0 = kvp.tile([128, W, NL], F8, tag="hT_full", name="hTf0")
            nc.sync.dma_start(hTf0[:, 0:4, :], xTf8[:, 0:4, :])
            nc.sync.dma_start(hTf0[:, 4:8, :], xTf8[:, 4:8, :])

            c_ln = [c_row[0:1, 0:768], c_row[0:1, 768:1536]]
            c_w2s = c_row[0:1, 1536:1664]
            c_lnn = [c_row[0:1, 1664:1792], c_row[0:1, 1792:1920]]
            r_g1n = c_row[0:1, 1920:2176]
            r_fb1 = c_row[0:1, 2176:2432]
            r_bv = c_row[0:1, 2432:2560]
            ones_r = c_row[0:1, 256:384]
            c_magic = wp.tile([128, 4], I32)
            nc.vector.memset(c_magic[:], RSQRT_MAGIC)
            c_ones1 = wp.tile([1, 64], F16)   # lhsT for den broadcast
            nc.vector.memset(c_ones1[:], 1.0)
            # warm the Act table set before the first real Act op needs it
            warm = wp.tile([1, 16], F32, name="warm")
            nc.vector.memset(warm[:], 0.0)
            nc.scalar.activation(warm[:], warm[:], AF.Exp)

            hT32 = sb.tile([128, NL], F32, tag="hT32")
            nc.sync.dma_start(hT32[:], xT32[:])

            # conv M operand: DMAs ride layer-2's AllGather window (idle DMA)
            m_t = hp.tile([128, 4, N], F16, name="m_t")

            def prefetch_conv_h(li, gate=None):
                if li == 1:
                    if gate is not None:
                        with nc.allow_low_precision(reason="dma gate"):
                            nc.vector.tensor_copy(m_t[0:1, 0:4, 0:1], gate)
                    for c in range(4):
                        nc.sync.dma_start(m_t[:, c, :], Mt[:, c, :])

            # ---------- helpers ----------
            def layer_norm(t32, t16_out, lnc, lncn, ff=False, t8_out=None,
                           t16_fast=False):
                """t32: [128, NL] f32 pre-LN input. Returns (h_ln f32, T strips,
                t16 fp16 copy of t32). T rows: [inv | m*inv] (+ [m | sigma]
                when ff=True, for the folded FF entry). Optionally writes an
                fp16 copy of the OUTPUT into t16_out."""
                t16 = sb.tile([128, NL], F16, tag="ln_t16")
                nc.vector.tensor_copy(t16[:], t32[:])
                t2 = sb.tile([128, NL], F16, tag="ln_t2")
                with nc.allow_low_precision(reason="LN sq to fp16"):
                    nc.scalar.square(t2[:], t32[:])
                stats = ps_m.tile([128, 8], F32, tag="m")
                for s in range(4):
                    nc.tensor.matmul(stats[:, s:s + 1],
                                     t16[:, s * 128:(s + 1) * 128], c_one16[:])
                    nc.tensor.matmul(stats[:, 4 + s:5 + s],
                                     t2[:, s * 128:(s + 1) * 128], c_one16[:])
                # stats already means (ones col = 1/128)
                m = stats[:, 0:4]
                msq = sb.tile([128, 4], F32, tag="ln_msq")
                nc.scalar.square(msq[:], m)
                ve = sb.tile([128, 4], F32, tag="ln_ve")
                nc.vector.scalar_tensor_tensor(ve[:], stats[:, 4:8], EPS, msq[:],
                                               ALU.add, ALU.subtract)
                # rsqrt via bit trick + 1 Newton iter (~0.2% rel err)
                sh = sb.tile([128, 4], I32, tag="ln_sh")
                nc.vector.tensor_scalar(sh[:], ve[:].bitcast(I32), 1, None,
                                        ALU.logical_shift_right)
                P = sb.tile([128, 8], F32, tag="ln_P")
                y = P[:, 0:4]
                nc.vector.tensor_tensor(y.bitcast(I32), c_magic[:], sh[:], ALU.subtract)
                a = sb.tile([128, 4], F32, tag="ln_a")
                nc.vector.tensor_tensor(a[:], y, y, ALU.mult)
                nc.vector.scalar_tensor_tensor(a[:], a[:], -0.5, ve[:],
                                               ALU.mult, ALU.mult)
                nc.vector.scalar_tensor_tensor(y, a[:], 1.5, y,
                                               ALU.add, ALU.mult)
                nc.vector.tensor_tensor(P[:, 4:8], m, y, ALU.mult)  # m*inv
                # strips to partition 0 via M=1 matmuls against identity:
                # out[0, j] = P16[j, s]
                nq = 4 if ff else 2
                P16 = sb.tile([128, 16], F16, tag="ln_P16")
                nc.vector.tensor_copy(P16[:, 0:8], P[:])
                if ff:
                    with nc.allow_low_precision(reason="LN strips fp16"):
                        nc.scalar.copy(P16[:, 8:12], m)
                        nc.vector.tensor_tensor(P16[:, 12:16], ve[:], y, ALU.mult)
                psT = [ps_s.tile([1, NL], F32, tag="scr", name=f"psT{q}")
                       for q in range(nq)]
                for q in range(nq):
                    for s in range(4):
                        nc.tensor.matmul(psT[q][:, s * 128:(s + 1) * 128],
                                         P16[:, 4 * q + s:4 * q + s + 1],
                                         c_id16[:])
                T = sb.tile([1, 2048], F16, tag="ln_T")
                with nc.allow_low_precision(reason="LN strip fp16"):
                    nc.scalar.copy(T[0:1, 0:NL], psT[0][:])
                    nc.vector.tensor_copy(T[0:1, NL:2 * NL], psT[1][:])
                    if ff:
                        nc.scalar.copy(T[0:1, 2 * NL:3 * NL], psT[2][:])
                        nc.vector.tensor_copy(T[0:1, 3 * NL:4 * NL], psT[3][:])
                # broadcast matmuls: A = g (x) inv ; B = (-g) (x) (m*inv) + b (x) 1
                psA = ps_m.tile([128, NL], F32, tag="m")
                psB = ps_m.tile([128, NL], F32, tag="m")
                nc.tensor.matmul(psA[:], lnc[0:1, 0:128], T[0:1, 0:NL])
                nc.tensor.matmul(psB[:], lncn[0:1, :], T[0:1, NL:2 * NL],
                                 start=True, stop=False)
                nc.tensor.matmul(psB[:], lnc[0:1, 128:256], lnc[0:1, 256:768],
                                 start=False, stop=True)
                u = sb.tile([128, NL], F32, tag="ln_u")
                nc.vector.tensor_tensor(u[:], t32[:], psA[:], ALU.mult)
                h_ln = sb.tile([128, NL], F32, tag="ln_out")
                if t8_out is not None:
                    # AG gate: fp8 output written first, directly from the add
                    with nc.allow_low_precision(reason="h8 for AllGather"):
                        nc.vector.tensor_tensor(t8_out[:], u[:], psB[:], ALU.add)
                nc.vector.tensor_tensor(h_ln[:], u[:], psB[:], ALU.add)
                if t16_out is not None:
                    with nc.allow_low_precision(reason="h16 copy"):
                        if t16_fast:
                            nc.vector.tensor_copy(t16_out[:], h_ln[:])
                        else:
                            nc.gpsimd.tensor_copy(t16_out[:], h_ln[:])
                return h_ln, T, t16

            # ---------- transformer layer ----------
            def mha_ff_layer(li, hT32_in, hT16_in, hT8_in, hTf_pre):
                # full h: layer 1 gets it free from the host input (pre-loaded);
                # layer 2 AllGathers the fp8 h produced by layer 1
                if hTf_pre is not None:
                    hTf = hTf_pre
                else:
                    hTf = kvp.tile([128, W, NL], F8, tag="hT_full")
                    nc.sync.dma_start(
                        kv_in[:].rearrange("(p j) -> p j", p=128), hT8_in[:])
                    nc.gpsimd.collective_compute(
                        "AllGather", ALU.bypass, replica_groups=RG,
                        ins=[kv_in[:]], outs=[kv_out[:]])

                ps_q = ps_m.tile([128, NL], F32, tag="m")
                nc.tensor.matmul(ps_q[:], w_q[:], hT16_in[:])
                # qz: fp8 q with a zero slot; scores use DoubleRow fp8 where
                # the k-side second slot is garbage annihilated by the zeros
                qz = sb.tile([128, 2, NL], F8, tag="qz")
                nc.gpsimd.memset(qz[:, 1, :], 0.0)
                with nc.allow_low_precision(reason="fp8 attention"):
                    nc.vector.tensor_scalar(qz[:, 0, :], ps_q[:], c_bq[:],
                                            None, ALU.add)

                # unpack h_full, then recompute kT/V locally (pipelines with attn)
                if hTf_pre is None:
                    kvv = kv_out[:].rearrange("r (p j) -> p r j", p=128)
                    nc.sync.dma_start(hTf[:, 0:1, :], kvv[:, 0:1, :])
                    nc.sync.dma_start(hTf[:, 1:4, :], kvv[:, 1:4, :])
                    nc.sync.dma_start(hTf[:, 4:8, :], kvv[:, 4:8, :])
                # kT8 layout [128, 5, 128]/rank: slots 0-3 = m-chunks, slot 4 =
                # pad so scores can view [64, 2, 128] slot pairs (cc, cc+1)
                kT8 = kvp.tile([128, W, 5, 128], F8, tag="kT_full")
                # per-head v padded to M=96: cols 0:64 v, 64 ones (denom),
                # 65:96 zero (DoubleRow needs M % 32 == 0, contiguous pairs,
                # dst partition base 0)
                vh = [kvp.tile([128, NCH, 96], F8, tag=f"v_h{h}",
                               name=f"vh{li}_{h}")
                      for h in range(H)]
                for h in range(H):
                    nc.gpsimd.memset(vh[h][:, :, 65:96], 0.0)
                    nc.gpsimd.memset(vh[h][:, :, 64:65], 1.0)
                # pad slot must be finite (NaN * 0 would poison DoubleRow)
                nc.gpsimd.memset(kT8[:, :, 4, :], 0.0)
                with nc.allow_low_precision(reason="fp8 attention"):
                    for r in range(W):
                        ps_k = ps_m.tile([128, 4, 128], F32, tag="m")
                        nc.tensor.matmul(ps_k[:], w_k[:], hTf[:, r, :])
                        if r % 2 == 0:
                            nc.scalar.activation(kT8[:, r, 0:4, :], ps_k[:],
                                                 AF.Identity, bias=c_bk[:])
                        else:
                            nc.vector.tensor_scalar(kT8[:, r, 0:4, :], ps_k[:],
                                                    c_bk[:], None, ALU.add)
                        for cp in range(2):
                            pv = ps_m.tile([128, 2, 128], F32, tag="m")
                            for j in range(2):
                                cc = 2 * cp + j
                                # bias prefilled via PE so the drain is a copy
                                nc.tensor.matmul(pv[:, j, :], ones_r[:],
                                                 r_bv[:], start=True, stop=False)
                                nc.tensor.matmul(
                                    pv[:, j, :],
                                    hTf[:, r, cc * 128:(cc + 1) * 128], w_v[:],
                                    start=False, stop=True)
                            c = 4 * r + 2 * cp
                            nc.vector.tensor_copy(vh[0][:, c:c + 2, 0:64],
                                                  pv[:, :, 0:64])
                            nc.scalar.copy(vh[1][:, c:c + 2, 0:64],
                                           pv[:, :, 64:128])

                prefetch_conv_h(li, gate=kT8[0:1, 0:1, 0:4, 0:1])

                # attention: fp8 DoubleRow scores + AV; exp split Act/DVE/Pool
                DR = mybir.MatmulPerfMode.DoubleRow
                S1, S2 = 1.4426950408889634, 54.76  # fp8e4 Schraudolph exp
                oT = sb.tile([128, NL], F16, tag="oT")
                po = [ps_o.tile([96, NL], F32, tag="o_acc", name=f"po{li}_{h}")
                      for h in range(H)]
                expi = 0
                for g in range(NCH // 2):
                    # ex_g[:, h, j, :]: head-major so AV reads a contiguous
                    # [128, 2, 512] pair per head; one exp op per chunk j
                    # covers both heads (strided output)
                    ex_g = expp.tile([128, H, 2, NL], I8, tag="exp",
                                     name=f"ex{li}_{g}")
                    for j in range(2):
                        c = 2 * g + j
                        r, cc = c // 4, c % 4
                        psc = ps_s.tile([128, H, NL], F32, tag="scr",
                                        name=f"scr{li}_{c}")
                        for h in range(H):
                            hs = slice(h * 64, (h + 1) * 64)
                            nc.tensor.matmul(
                                psc[:, h, :],
                                kT8[hs, r, cc:cc + 2, :],
                                qz[hs, :, :], perf_mode=DR)
                        eng = EXPSPLIT[expi % len(EXPSPLIT)]
                        expi += 1
                        with nc.allow_low_precision(reason="fp8 exp"):
                            if eng == "A":
                                nc.scalar.activation(
                                    ex_g[:, :, j, :].bitcast(F8), psc[:],
                                    AF.Exp, scale=0.125)
                            else:
                                nc.vector.tensor_scalar(ex_g[:, :, j, :],
                                                        psc[:], S1, S2,
                                                        ALU.mult, ALU.add)
                    for h in range(H):
                        exr = ex_g[:, h, :, :].bitcast(F8)
                        nc.tensor.matmul(
                            po[h][:], vh[h][:, 2 * g:2 * g + 2, :], exr,
                            start=(g == 0), stop=(g == NCH // 2 - 1),
                            perf_mode=DR)
                for h in range(H):
                    hs = slice(h * 64, (h + 1) * 64)
                    # normalize: fast recip of den row, broadcast, multiply
                    den32 = sb.tile([1, NL], F32, tag="den32")
                    if h == 0:
                        nc.vector.tensor_copy(den32[:], po[h][64:65, :])
                    else:
                        nc.scalar.copy(den32[:], po[h][64:65, :])
                    rden32 = sb.tile([1, NL], F32, tag="rden32")
                    nc.vector.reciprocal_approx_fast(rden32[:], den32[:])
                    rden = sb.tile([1, NL], F16, tag="rden")
                    with nc.allow_low_precision(reason="attn denom recip fp16"):
                        nc.scalar.copy(rden[:], rden32[:])
                    pden = ps_m.tile([64, NL], F32, tag="m")
                    nc.tensor.matmul(pden[:], c_ones1[:], rden[:])
                    denB = sb.tile([64, NL], F16, tag="denB")
                    with nc.allow_low_precision(reason="denB fp16"):
                        if h == 0:
                            nc.vector.tensor_copy(denB[:], pden[:])
                        else:
                            nc.scalar.copy(denB[:], pden[:])
                    nc.vector.tensor_tensor(oT[hs, :], po[h][0:64, :], denB[:],
                                            ALU.mult)

                # o-projection + residual
                ps_p = ps_m.tile([128, NL], F32, tag="m")
                nc.tensor.matmul(ps_p[:], w_o[:], oT[:])
                t1 = sb.tile([128, NL], F32, tag="resid1")
                nc.vector.scalar_tensor_tensor(t1[:], ps_p[:], c_bo[:], hT32_in[:],
                                               ALU.add, ALU.add)

                h1_32, T1, t1_16 = layer_norm(t1, None, c_ln[0], c_lnn[0],
                                              ff=True)

                # FF folded through LN1: tanh arg = inv (.) [W1G^T t1
                # + G1N (x) m + FB1 (x) sigma]; sigmoid affine folded into
                # host-scaled W2 (0.5*W2) plus const row 0.5*colsum(W2)
                psI = ps_s.tile([128, NL], F32, tag="scr", name="psI")
                nc.tensor.matmul(psI[:], c_ln[0][0:1, 256:384], T1[0:1, 0:NL])
                invB = sb.tile([128, NL], F16, tag="invB")
                with nc.allow_low_precision(reason="invB fp16"):
                    nc.scalar.copy(invB[:], psI[:])
                z = sb.tile([128, 2, NL], F16, tag="z")
                zarg = sb.tile([128, 2, NL], F16, tag="zarg")
                for f in range(2):
                    pz = ps_m.tile([128, NL], F32, tag="m")
                    nc.tensor.matmul(pz[:], w_1g[:, f * 128:(f + 1) * 128],
                                     t1_16[:], start=True, stop=False)
                    nc.tensor.matmul(pz[:], r_g1n[0:1, f * 128:(f + 1) * 128],
                                     T1[0:1, 2 * NL:3 * NL],
                                     start=False, stop=False)
                    nc.tensor.matmul(pz[:], r_fb1[0:1, f * 128:(f + 1) * 128],
                                     T1[0:1, 3 * NL:4 * NL],
                                     start=False, stop=True)
                    with nc.allow_low_precision(reason="tanh arg fp16"):
                        nc.vector.tensor_tensor(zarg[:, f, :], pz[:], invB[:],
                                                ALU.mult)
                ps_f = ps_m.tile([128, NL], F32, tag="m")
                nc.tensor.matmul(ps_f[:], c_w2s[:], c_ln[0][0:1, 256:768],
                                 start=True, stop=False)
                for f in range(2):
                    nc.scalar.activation(z[:, f, :], zarg[:, f, :], AF.Tanh)
                    nc.tensor.matmul(ps_f[:], w2sl(f), z[:, f, :],
                                     start=False, stop=(f == 1))
                t2 = sb.tile([128, NL], F32, tag="resid2")
                nc.vector.scalar_tensor_tensor(t2[:], ps_f[:], c_bf2[:], h1_32[:],
                                               ALU.add, ALU.add)

                h2_16 = sb.tile([128, NL], F16, tag="hT16")
                h2_8 = None
                if li == 0:
                    h2_8 = sb.tile([128, NL], F8, tag="hT8")
                h2_32, _, _ = layer_norm(t2, h2_16, c_ln[1], c_lnn[1],
                                         t8_out=h2_8, t16_fast=(li == 1))
                return h2_32, h2_16, h2_8

            h32, h16, h8 = mha_ff_layer(0, hT32, hT16, None, hTf0)
            h32, h16, _ = mha_ff_layer(1, h32, h16, h8, None)

            # ---------- hypergraph conv ----------
            # out = relu((M @ h) @ Wh + bh), M = D^-1 H B^-1 H^T precomputed
            # on host (Dinv folded into Mt columns). Each core computes
            # pT[:, block b] = h_loc^T-partials, then one ReduceScatter sums
            # partials across cores and leaves each core its own node block.
            h_nat = sb.tile([128, 4, 128], F16, tag="h_nat")
            for c in range(4):
                pt = ps_m.tile([128, 128], F16, tag="m")
                nc.tensor.transpose(pt[:], h16[:, c * 128:(c + 1) * 128],
                                    c_id16[:])
                nc.vector.tensor_copy(h_nat[:, c, :], pt[:])
            rs_sb = sb.tile([128, W, NL], F16, tag="rs_sb")
            for b in range(W):
                pb = ps_s.tile([128, NL], F32, tag="scr", name=f"pconv{b}")
                for c in range(4):
                    nc.tensor.matmul(pb[:], h_nat[:, c, :],
                                     m_t[:, c, b * NL:(b + 1) * NL],
                                     start=(c == 0), stop=(c == 3))
                with nc.allow_low_precision(reason="conv partial fp16"):
                    if b % 2 == 0:
                        nc.vector.tensor_copy(rs_sb[:, b, :], pb[:])
                    else:
                        nc.scalar.copy(rs_sb[:, b, :], pb[:])
                nc.sync.dma_start(rs_in[b, :, :], rs_sb[:, b, :])
            nc.gpsimd.collective_compute(
                "ReduceScatter", ALU.add, replica_groups=RG,
                ins=[rs_in[:]], outs=[rs_out[:]])
            pT_loc = sb.tile([128, NL], F16, tag="pT_loc")
            nc.sync.dma_start(pT_loc[:], rs_out[:])
            po2 = ps_m.tile([128, NL], F32, tag="m")
            nc.tensor.matmul(po2[:], w_h[:], pT_loc[:])
            res = sb.tile([128, NL], F32, tag="res")
            nc.scalar.activation(res[:], po2[:], AF.Relu, bias=c_bh[:])
            nc.sync.dma_start(out_t[:], res[:])

    nc.compile()
    return nc


_NC = None


def _get_nc():
    global _NC
    if _NC is None:
        _NC = build_program()
    return _NC


def make_in_maps(inputs):
    x = np.asarray(inputs["x"], dtype=np.float32)
    edge = np.asarray(inputs["edge"])
    gw = {k: np.asarray(inputs[k], dtype=np.float32) for k in
          ("Wq", "bq", "Wk", "bk", "Wv", "bv", "Wo", "bo", "g_ln1", "b_ln1",
           "W1", "bf1", "W2", "bf2", "g_ln2", "b_ln2", "Wh", "bh")}

    node_idx = np.asarray(edge[0], dtype=np.int64)
    he_idx = np.asarray(edge[1], dtype=np.int64)
    counts = np.zeros((N, NE), dtype=np.float32)
    np.add.at(counts, (node_idx, he_idx), 1.0)
    Bdeg = counts.sum(axis=0)
    Ddeg = counts.sum(axis=1)
    Binv = np.where(Bdeg > 0, 1.0 / np.maximum(Bdeg, 1), 0.0).astype(np.float32)
    Dinv = np.where(Ddeg > 0, 1.0 / np.maximum(Ddeg, 1), 0.0).astype(np.float32)

    # S = H B^-1 H^T (symmetric, [N, N]); conv operand per core r:
    # Mt_r[jl, n] = S[loc_r(jl), n] * Dinv[n]
    from scipy import sparse
    Hs = sparse.csr_matrix(counts)
    S = np.asarray((Hs.multiply(Binv[None, :]) @ Hs.T).todense(),
                   dtype=np.float32)
    SD = S * Dinv[None, :]

    # CW16 pack: Wkv | Wq | Wo | W1G | Wh | id16 | w2(0.5) | ones/128
    cw16 = np.zeros((128, 1283), dtype=np.float16)
    cw16[:, 0:128] = gw["Wk"].astype(np.float16)
    cw16[:, 128:256] = gw["Wv"].astype(np.float16)
    cw16[:, 258:386] = gw["Wq"].astype(np.float16)
    cw16[:, 386:514] = gw["Wo"].astype(np.float16)
    w1g = 0.5 * gw["W1"] * gw["g_ln1"][:, None]
    cw16[:, 514:770] = w1g.astype(np.float16)
    cw16[:, 770:898] = gw["Wh"].astype(np.float16)
    cw16[:, 898:1026] = np.eye(128, dtype=np.float16)
    cw16[:, 1026:1282] = np.ascontiguousarray(
        (0.5 * gw["W2"]).reshape(2, 128, 128).transpose(1, 0, 2)
    ).reshape(128, 256).astype(np.float16)
    cw16[:, 1282] = 1.0 / 128

    # CW32 pack: bkv(131) | bq | bo | bf2 | bh
    cw32 = np.zeros((128, 135), dtype=np.float32)
    cw32[:, 0] = gw["bk"]
    cw32[:, 1:65] = gw["bv"][0:64][None, :]
    cw32[:, 66:130] = gw["bv"][64:128][None, :]
    cw32[:, 65] = 1.0
    cw32[:, 130] = 1.0
    cw32[:, 131] = gw["bq"]
    cw32[:, 132] = gw["bo"]
    cw32[:, 133] = gw["bf2"]
    cw32[:, 134] = gw["bh"]

    def lnrows(g, b):
        r = np.zeros((768,), dtype=np.float16)
        r[0:128] = g
        r[128:256] = b
        r[256:768] = 1.0
        return r

    # CROW pack: ln1r | ln2r | w2sum | ln1rn | ln2rn | g1n | fb1
    crow = np.zeros((1, 2562), dtype=np.float16)
    crow[0, 0:768] = lnrows(gw["g_ln1"], gw["b_ln1"])
    crow[0, 768:1536] = lnrows(gw["g_ln2"], gw["b_ln2"])
    crow[0, 1536:1664] = (0.5 * gw["W2"].sum(axis=0)).astype(np.float16)
    crow[0, 1664:1792] = (-gw["g_ln1"]).astype(np.float16)
    crow[0, 1792:1920] = (-gw["g_ln2"]).astype(np.float16)
    crow[0, 1920:2176] = (-0.5 * (gw["W1"].T @ gw["g_ln1"])).astype(np.float16)
    crow[0, 2176:2432] = (0.5 * (gw["W1"].T @ gw["b_ln1"])
                          + 0.5 * gw["bf1"]).astype(np.float16)
    crow[0, 2432:2560] = gw["bv"].astype(np.float16)

    shared = dict(CW16=cw16, CW32=cw32, CROW=crow)

    import ml_dtypes
    F8NP = ml_dtypes.float8_e4m3
    xTf = np.ascontiguousarray(
        x.T.reshape(128, W, NL)).astype(F8NP)
    in_maps = []
    for r in range(W):
        rows = slice(r * NL, (r + 1) * NL)
        xT = np.ascontiguousarray(x[rows, :].T)
        mt = np.ascontiguousarray(
            SD[rows, :].reshape(4, 128, N).transpose(1, 0, 2).astype(np.float16))
        m = dict(shared)
        m.update(xTf8=xTf, xT32=xT.astype(np.float32), xT16=xT.astype(np.float16),
                 Mt=mt)
        in_maps.append(m)
    return in_maps


def kernel(**inputs) -> np.ndarray:
    nc = _get_nc()
    in_maps = make_in_maps(inputs)
    res = run_bass_kernel_spmd(nc, in_maps, core_ids=list(range(W)))
    out = np.empty((N, D), dtype=np.float32)
    for r in range(W):
        out[r * NL:(r + 1) * NL, :] = res.results[r]["outT"].T
    return out


if __name__ == "__main__":
    build_program()
    print("build OK")



# revision 86
# speedup vs baseline: 1.0066x; 1.0001x over previous
"""Trainium2 Bass kernel for nn_CIE_89893665505337 (gnn_message_passing).

2x [MHA(global over 4096 nodes) + FF] transformer blocks + HypergraphConv.
8-core SPMD: nodes sharded 512/core, transposed activations hT [D=128, n],
fp16 matmul operands, f32 residual stream, AllGather for K/V, xt, e.
"""
import os
import sys

for _p in ("/opt/trn_rl_repo", "/root/.axon_site/_ro/trn_rl_repo"):
    if os.path.isdir(_p) and _p not in sys.path:
        sys.path.insert(0, _p)

import numpy as np

import concourse.bacc as bacc
import concourse.bass as bass
import concourse.tile as tile
from concourse import mybir
from concourse.bass_utils import run_bass_kernel_spmd

F32 = mybir.dt.float32
F16 = mybir.dt.float16
I32 = mybir.dt.int32
I8 = mybir.dt.int8
# exp engine split (Act / DVE only: gpsimd has no PSUM port); 32 ops/layer
EXPSPLIT = "AD"
AF = mybir.ActivationFunctionType
ALU = mybir.AluOpType

W = 8            # cores
N = 4096         # nodes
D = 128          # model dim
H = 2            # heads
DH = 64          # head dim
FF = 256         # ff dim
NE = 2048        # hyperedges
NL = N // W      # 512 local nodes
EL = NE // W     # 256 local hyperedges
EPS = 1e-5
NCH = N // 128   # 32 m-chunks
RSQRT_MAGIC = 0x5F3759DF

H_SZ = 128 * NL            # fp16 elems of hT_loc [128, 512]


def build_program():
    nc = bacc.Bacc("TRN2", target_bir_lowering=False, debug=False, num_devices=W)

    def inp(name, shape, dt=F32):
        return nc.dram_tensor(name, shape, dt, kind="ExternalInput")

    F8 = mybir.dt.float8e4
    # per-core inputs
    xT32 = inp("xT32", [128, NL])
    xT16 = inp("xT16", [128, NL], F16)
    xTf8 = inp("xTf8", [128, W, NL], F8)      # full x^T, rank-blocked, fp8
    # packed weights (one DMA each):
    # CW16: Wkv 0:258 | Wq 258:386 | Wo 386:514 | W1G 514:770 | Wh 770:898
    #       | id16 898:1026 | w2 1026:1282 | ones/128 1282:1283
    CW16 = inp("CW16", [128, 1283], F16)
    # CW32: bkv 0:131 | bq | bo | bf2 | bh
    CW32 = inp("CW32", [128, 135])
    # CROW: ln1r 0:768 | ln2r 768:1536 | w2sum 1536:1664 | ln1rn 1664:1792
    #       | ln2rn 1792:1920 | g1n 1920:2176 | fb1 2176:2432
    CROW = inp("CROW", [1, 2562], F16)
    # conv operand: Mt[p, c, n] = S[loc(c*128+p), n] * Dinv[n]  (fp16)
    Mt = inp("Mt", [128, 4, N], F16)

    out_t = nc.dram_tensor("outT", [128, NL], F32, kind="ExternalOutput")

    # AG bounce tensors (internal DRAM); outputs Shared
    kv_in = nc.dram_tensor("kv_in1", [H_SZ], F8)
    kv_out = nc.dram_tensor("kv_out1", [W, H_SZ], F8, addr_space="Shared")
    # conv ReduceScatter bounce
    rs_in = nc.dram_tensor("rs_in", [W, 128, NL], F16)
    rs_out = nc.dram_tensor("rs_out", [128, NL], F16)

    RG = [list(range(W))]

    with tile.TileContext(nc) as tc:
        with (
            tc.tile_pool(name="wpool", bufs=1) as wp,      # persistent weights/consts
            tc.tile_pool(name="sb", bufs=3) as sb,         # general sbuf tiles
            tc.tile_pool(name="kv", bufs=2) as kvp,        # kT/V per layer
            tc.tile_pool(name="expp", bufs=4) as expp,     # exp tiles
            tc.tile_pool(name="hp", bufs=1) as hp,         # conv H tiles
            tc.tile_pool(name="ps_s", bufs=2, space="PSUM") as ps_s,   # scores 2 banks each
            tc.tile_pool(name="ps_o", bufs=2, space="PSUM") as ps_o,   # attn out acc
            tc.tile_pool(name="ps_m", bufs=2, space="PSUM") as ps_m,   # misc
        ):
            # ---- load constants ----
            def load(name, shape, dram, dt=F32):
                t = wp.tile(shape, dt, name=name)
                nc.sync.dma_start(t[:], dram[:])
                return t

            F8 = mybir.dt.float8e4
            # critical path first: packed weights, then layer-1 inputs
            c_w16 = wp.tile([128, 1283], F16, name="c_w16")
            nc.gpsimd.dma_start(c_w16[:], CW16[:])
            c_w32 = wp.tile([128, 135], F32, name="c_w32")
            nc.gpsimd.dma_start(c_w32[:], CW32[:])
            hT16 = sb.tile([128, NL], F16, tag="hT16")
            nc.gpsimd.dma_start(hT16[:], xT16[:])
            c_row = wp.tile([1, 2562], F16, name="c_row")
            nc.gpsimd.dma_start(c_row[:], CROW[:])
            w_kv = c_w16
            w_k = c_w16[:, 0:128]
            w_v = c_w16[:, 128:256]
            w_q = c_w16[:, 258:386]
            w_o = c_w16[:, 386:514]
            w_1g = c_w16[:, 514:770]
            w_h = c_w16[:, 770:898]
            c_id16 = c_w16[:, 898:1026]
            c_one16 = c_w16[:, 1282:1283]
            c_bk = c_w32[:, 0:1]
            c_bq = c_w32[:, 131:132]
            c_bo = c_w32[:, 132:133]
            c_bf2 = c_w32[:, 133:134]
            c_bh = c_w32[:, 134:135]

            def w2sl(cix):
                return c_w16[:, 1026 + 128 * cix:1026 + 128 * (cix + 1)]

            # layer-1 full-x load: highest DMA priority (attention-1 gate)
            hTf0 = kvp.tile([128, W, NL], F8, tag="hT_full", name="hTf0")
            nc.sync.dma_start(hTf0[:, 0:4, :], xTf8[:, 0:4, :])
            nc.sync.dma_start(hTf0[:, 4:8, :], xTf8[:, 4:8, :])

            c_ln = [c_row[0:1, 0:768], c_row[0:1, 768:1536]]
            c_w2s = c_row[0:1, 1536:1664]
            c_lnn = [c_row[0:1, 1664:1792], c_row[0:1, 1792:1920]]
            r_g1n = c_row[0:1, 1920:2176]
            r_fb1 = c_row[0:1, 2176:2432]
            r_bv = c_row[0:1, 2432:2560]
            ones_r = c_row[0:1, 256:384]
            c_magic = wp.tile([128, 4], I32)
            nc.vector.memset(c_magic[:], RSQRT_MAGIC)
            c_ones1 = wp.tile([1, 64], F16)   # lhsT for den broadcast
            nc.vector.memset(c_ones1[:], 1.0)
            # warm the Act table set before the first real Act op needs it
            warm = wp.tile([1, 16], F32, name="warm")
            nc.vector.memset(warm[:], 0.0)
            nc.scalar.activation(warm[:], warm[:], AF.Exp)

            hT32 = sb.tile([128, NL], F32, tag="hT32")
            nc.sync.dma_start(hT32[:], xT32[:])

            # conv M operand: DMAs ride layer-2's AllGather window (idle DMA)
            m_t = hp.tile([128, 4, N], F16, name="m_t")

            def prefetch_conv_h(li, gate=None):
                if li == 1:
                    if gate is not None:
                        with nc.allow_low_precision(reason="dma gate"):
                            nc.vector.tensor_copy(m_t[0:1, 0:4, 0:1], gate)
                    for c in range(4):
                        nc.sync.dma_start(m_t[:, c, :], Mt[:, c, :])

            # ---------- helpers ----------
            def layer_norm(t32, t16_out, lnc, lncn, ff=False, t8_out=None,
                           t16_fast=False):
                """t32: [128, NL] f32 pre-LN input. Returns (h_ln f32, T strips,
                t16 fp16 copy of t32). T rows: [inv | m*inv] (+ [m | sigma]
                when ff=True, for the folded FF entry). Optionally writes an
                fp16 copy of the OUTPUT into t16_out."""
                t16 = sb.tile([128, NL], F16, tag="ln_t16")
                nc.vector.tensor_copy(t16[:], t32[:])
                t2 = sb.tile([128, NL], F16, tag="ln_t2")
                with nc.allow_low_precision(reason="LN sq to fp16"):
                    nc.scalar.square(t2[:], t32[:])
                stats = ps_m.tile([128, 8], F32, tag="m")
                for s in range(4):
                    nc.tensor.matmul(stats[:, s:s + 1],
                                     t16[:, s * 128:(s + 1) * 128], c_one16[:])
                    nc.tensor.matmul(stats[:, 4 + s:5 + s],
                                     t2[:, s * 128:(s + 1) * 128], c_one16[:])
                # stats already means (ones col = 1/128)
                m = stats[:, 0:4]
                msq = sb.tile([128, 4], F32, tag="ln_msq")
                nc.scalar.square(msq[:], m)
                ve = sb.tile([128, 4], F32, tag="ln_ve")
                nc.vector.scalar_tensor_tensor(ve[:], stats[:, 4:8], EPS, msq[:],
                                               ALU.add, ALU.subtract)
                # rsqrt via bit trick + 1 Newton iter (~0.2% rel err)
                sh = sb.tile([128, 4], I32, tag="ln_sh")
                nc.vector.tensor_scalar(sh[:], ve[:].bitcast(I32), 1, None,
                                        ALU.logical_shift_right)
                P = sb.tile([128, 8], F32, tag="ln_P")
                y = P[:, 0:4]
                nc.vector.tensor_tensor(y.bitcast(I32), c_magic[:], sh[:], ALU.subtract)
                a = sb.tile([128, 4], F32, tag="ln_a")
                nc.vector.tensor_tensor(a[:], y, y, ALU.mult)
                nc.vector.scalar_tensor_tensor(a[:], a[:], -0.5, ve[:],
                                               ALU.mult, ALU.mult)
                nc.vector.scalar_tensor_tensor(y, a[:], 1.5, y,
                                               ALU.add, ALU.mult)
                nc.vector.tensor_tensor(P[:, 4:8], m, y, ALU.mult)  # m*inv
                # strips to partition 0 via M=1 matmuls against identity:
                # out[0, j] = P16[j, s]
                nq = 4 if ff else 2
                P16 = sb.tile([128, 16], F16, tag="ln_P16")
                nc.vector.tensor_copy(P16[:, 0:8], P[:])
                if ff:
                    with nc.allow_low_precision(reason="LN strips fp16"):
                        nc.scalar.copy(P16[:, 8:12], m)
                        nc.vector.tensor_tensor(P16[:, 12:16], ve[:], y, ALU.mult)
                psT = [ps_s.tile([1, NL], F32, tag="scr", name=f"psT{q}")
                       for q in range(nq)]
                for q in range(nq):
                    for s in range(4):
                        nc.tensor.matmul(psT[q][:, s * 128:(s + 1) * 128],
                                         P16[:, 4 * q + s:4 * q + s + 1],
                                         c_id16[:])
                T = sb.tile([1, 2048], F16, tag="ln_T")
                with nc.allow_low_precision(reason="LN strip fp16"):
                    nc.scalar.copy(T[0:1, 0:NL], psT[0][:])
                    nc.vector.tensor_copy(T[0:1, NL:2 * NL], psT[1][:])
                    if ff:
                        nc.scalar.copy(T[0:1, 2 * NL:3 * NL], psT[2][:])
                        nc.vector.tensor_copy(T[0:1, 3 * NL:4 * NL], psT[3][:])
                # broadcast matmuls: A = g (x) inv ; B = (-g) (x) (m*inv) + b (x) 1
                psA = ps_m.tile([128, NL], F32, tag="m")
                psB = ps_m.tile([128, NL], F32, tag="m")
                nc.tensor.matmul(psA[:], lnc[0:1, 0:128], T[0:1, 0:NL])
                nc.tensor.matmul(psB[:], lncn[0:1, :], T[0:1, NL:2 * NL],
                                 start=True, stop=False)
                nc.tensor.matmul(psB[:], lnc[0:1, 128:256], lnc[0:1, 256:768],
                                 start=False, stop=True)
                u = sb.tile([128, NL], F32, tag="ln_u")
                nc.vector.tensor_tensor(u[:], t32[:], psA[:], ALU.mult)
                h_ln = sb.tile([128, NL], F32, tag="ln_out")
                if t8_out is not None:
                    # AG gate: fp8 output written first, directly from the add
                    with nc.allow_low_precision(reason="h8 for AllGather"):
                        nc.vector.tensor_tensor(t8_out[:], u[:], psB[:], ALU.add)
                nc.vector.tensor_tensor(h_ln[:], u[:], psB[:], ALU.add)
                if t16_out is not None:
                    with nc.allow_low_precision(reason="h16 copy"):
                        if t16_fast:
                            nc.vector.tensor_copy(t16_out[:], h_ln[:])
                        else:
                            nc.gpsimd.tensor_copy(t16_out[:], h_ln[:])
                return h_ln, T, t16

            # ---------- transformer layer ----------
            def mha_ff_layer(li, hT32_in, hT16_in, hT8_in, hTf_pre):
                # full h: layer 1 gets it free from the host input (pre-loaded);
                # layer 2 AllGathers the fp8 h produced by layer 1
                if hTf_pre is not None:
                    hTf = hTf_pre
                else:
                    hTf = kvp.tile([128, W, NL], F8, tag="hT_full")
                    nc.sync.dma_start(
                        kv_in[:].rearrange("(p j) -> p j", p=128), hT8_in[:])
                    nc.gpsimd.collective_compute(
                        "AllGather", ALU.bypass, replica_groups=RG,
                        ins=[kv_in[:]], outs=[kv_out[:]])

                ps_q = ps_m.tile([128, NL], F32, tag="m")
                nc.tensor.matmul(ps_q[:], w_q[:], hT16_in[:])
                # qz: fp8 q with a zero slot; scores use DoubleRow fp8 where
                # the k-side second slot is garbage annihilated by the zeros
                qz = sb.tile([128, 2, NL], F8, tag="qz")
                nc.gpsimd.memset(qz[:, 1, :], 0.0)
                with nc.allow_low_precision(reason="fp8 attention"):
                    nc.vector.tensor_scalar(qz[:, 0, :], ps_q[:], c_bq[:],
                                            None, ALU.add)

                # unpack h_full, then recompute kT/V locally (pipelines with attn)
                if hTf_pre is None:
                    kvv = kv_out[:].rearrange("r (p j) -> p r j", p=128)
                    nc.sync.dma_start(hTf[:, 0:1, :], kvv[:, 0:1, :])
                    nc.sync.dma_start(hTf[:, 1:4, :], kvv[:, 1:4, :])
                    nc.sync.dma_start(hTf[:, 4:8, :], kvv[:, 4:8, :])
                # kT8 layout [128, 5, 128]/rank: slots 0-3 = m-chunks, slot 4 =
                # pad so scores can view [64, 2, 128] slot pairs (cc, cc+1)
                kT8 = kvp.tile([128, W, 5, 128], F8, tag="kT_full")
                # per-head v padded to M=96: cols 0:64 v, 64 ones (denom),
                # 65:96 zero (DoubleRow needs M % 32 == 0, contiguous pairs,
                # dst partition base 0)
                vh = [kvp.tile([128, NCH, 96], F8, tag=f"v_h{h}",
                               name=f"vh{li}_{h}")
                      for h in range(H)]
                for h in range(H):
                    nc.gpsimd.memset(vh[h][:, :, 65:96], 0.0)
                    nc.gpsimd.memset(vh[h][:, :, 64:65], 1.0)
                # pad slot must be finite (NaN * 0 would poison DoubleRow)
                nc.gpsimd.memset(kT8[:, :, 4, :], 0.0)
                with nc.allow_low_precision(reason="fp8 attention"):
                    for r in range(W):
                        ps_k = ps_m.tile([128, 4, 128], F32, tag="m")
                        nc.tensor.matmul(ps_k[:], w_k[:], hTf[:, r, :])
                        if r % 2 == 0:
                            nc.scalar.activation(kT8[:, r, 0:4, :], ps_k[:],
                                                 AF.Identity, bias=c_bk[:])
                        else:
                            nc.vector.tensor_scalar(kT8[:, r, 0:4, :], ps_k[:],
                                                    c_bk[:], None, ALU.add)
                        for cp in range(2):
                            pv = ps_m.tile([128, 2, 128], F32, tag="m")
                            for j in range(2):
                                cc = 2 * cp + j
                                # bias prefilled via PE so the drain is a copy
                                nc.tensor.matmul(pv[:, j, :], ones_r[:],
                                                 r_bv[:], start=True, stop=False)
                                nc.tensor.matmul(
                                    pv[:, j, :],
                                    hTf[:, r, cc * 128:(cc + 1) * 128], w_v[:],
                                    start=False, stop=True)
                            c = 4 * r + 2 * cp
                            nc.vector.tensor_copy(vh[0][:, c:c + 2, 0:64],
                                                  pv[:, :, 0:64])
                            nc.scalar.copy(vh[1][:, c:c + 2, 0:64],
                                           pv[:, :, 64:128])

                prefetch_conv_h(li, gate=kT8[0:1, 0:1, 0:4, 0:1])

                # attention: fp8 DoubleRow scores + AV; exp split Act/DVE/Pool
                DR = mybir.MatmulPerfMode.DoubleRow
                S1, S2 = 1.4426950408889634, 54.76  # fp8e4 Schraudolph exp
                oT = sb.tile([128, NL], F16, tag="oT")
                po = [ps_o.tile([96, NL], F32, tag="o_acc", name=f"po{li}_{h}")
                      for h in range(H)]
                expi = 0
                for g in range(NCH // 2):
                    # ex_g[:, h, j, :]: head-major so AV reads a contiguous
                    # [128, 2, 512] pair per head; one exp op per chunk j
                    # covers both heads (strided output)
                    ex_g = expp.tile([128, H, 2, NL], I8, tag="exp",
                                     name=f"ex{li}_{g}")
                    for j in range(2):
                        c = 2 * g + j
                        r, cc = c // 4, c % 4
                        psc = ps_s.tile([128, H, NL], F32, tag="scr",
                                        name=f"scr{li}_{c}")
                        for h in range(H):
                            hs = slice(h * 64, (h + 1) * 64)
                            nc.tensor.matmul(
                                psc[:, h, :],
                                kT8[hs, r, cc:cc + 2, :],
                                qz[hs, :, :], perf_mode=DR)
                        eng = EXPSPLIT[expi % len(EXPSPLIT)]
                        expi += 1
                        with nc.allow_low_precision(reason="fp8 exp"):
                            if eng == "A":
                                nc.scalar.activation(
                                    ex_g[:, :, j, :].bitcast(F8), psc[:],
                                    AF.Exp, scale=0.125)
                            else:
                                nc.vector.tensor_scalar(ex_g[:, :, j, :],
                                                        psc[:], S1, S2,
                                                        ALU.mult, ALU.add)
                    for h in range(H):
                        exr = ex_g[:, h, :, :].bitcast(F8)
                        nc.tensor.matmul(
                            po[h][:], vh[h][:, 2 * g:2 * g + 2, :], exr,
                            start=(g == 0), stop=(g == NCH // 2 - 1),
                            perf_mode=DR)
                for h in range(H):
                    hs = slice(h * 64, (h + 1) * 64)
                    # normalize: fast recip of den row, broadcast, multiply
                    den32 = sb.tile([1, NL], F32, tag="den32")
                    if h == 0:
                        nc.vector.tensor_copy(den32[:], po[h][64:65, :])
                    else:
                        nc.scalar.copy(den32[:], po[h][64:65, :])
                    rden32 = sb.tile([1, NL], F32, tag="rden32")
                    nc.vector.reciprocal_approx_fast(rden32[:], den32[:])
                    rden = sb.tile([1, NL], F16, tag="rden")
                    with nc.allow_low_precision(reason="attn denom recip fp16"):
                        nc.scalar.copy(rden[:], rden32[:])
                    pden = ps_m.tile([64, NL], F32, tag="m")
                    nc.tensor.matmul(pden[:], c_ones1[:], rden[:])
                    denB = sb.tile([64, NL], F16, tag="denB")
                    with nc.allow_low_precision(reason="denB fp16"):
                        if h == 0:
                            nc.vector.tensor_copy(denB[:], pden[:])
                        else:
                            nc.scalar.copy(denB[:], pden[:])
                    nc.vector.tensor_tensor(oT[hs, :], po[h][0:64, :], denB[:],
                                            ALU.mult)

                # o-projection + residual
                ps_p = ps_m.tile([128, NL], F32, tag="m")
                nc.tensor.matmul(ps_p[:], w_o[:], oT[:])
                t1 = sb.tile([128, NL], F32, tag="resid1")
                nc.vector.scalar_tensor_tensor(t1[:], ps_p[:], c_bo[:], hT32_in[:],
                                               ALU.add, ALU.add)

                h1_32, T1, t1_16 = layer_norm(t1, None, c_ln[0], c_lnn[0],
                                              ff=True)

                # FF folded through LN1: tanh arg = inv (.) [W1G^T t1
                # + G1N (x) m + FB1 (x) sigma]; sigmoid affine folded into
                # host-scaled W2 (0.5*W2) plus const row 0.5*colsum(W2)
                psI = ps_s.tile([128, NL], F32, tag="scr", name="psI")
                nc.tensor.matmul(psI[:], c_ln[0][0:1, 256:384], T1[0:1, 0:NL])
                invB = sb.tile([128, NL], F16, tag="invB")
                with nc.allow_low_precision(reason="invB fp16"):
                    nc.scalar.copy(invB[:], psI[:])
                z = sb.tile([128, 2, NL], F16, tag="z")
                zarg = sb.tile([128, 2, NL], F16, tag="zarg")
                for f in range(2):
                    pz = ps_m.tile([128, NL], F32, tag="m")
                    nc.tensor.matmul(pz[:], w_1g[:, f * 128:(f + 1) * 128],
                                     t1_16[:], start=True, stop=False)
                    nc.tensor.matmul(pz[:], r_g1n[0:1, f * 128:(f + 1) * 128],
                                     T1[0:1, 2 * NL:3 * NL],
                                     start=False, stop=False)
                    nc.tensor.matmul(pz[:], r_fb1[0:1, f * 128:(f + 1) * 128],
                                     T1[0:1, 3 * NL:4 * NL],
                                     start=False, stop=True)
                    with nc.allow_low_precision(reason="tanh arg fp16"):
                        nc.vector.tensor_tensor(zarg[:, f, :], pz[:], invB[:],
                                                ALU.mult)
                ps_f = ps_m.tile([128, NL], F32, tag="m")
                nc.tensor.matmul(ps_f[:], c_w2s[:], c_ln[0][0:1, 256:768],
                                 start=True, stop=False)
                for f in range(2):
                    nc.scalar.activation(z[:, f, :], zarg[:, f, :], AF.Tanh)
                    nc.tensor.matmul(ps_f[:], w2sl(f), z[:, f, :],
                                     start=False, stop=(f == 1))
                t2 = sb.tile([128, NL], F32, tag="resid2")
                nc.vector.scalar_tensor_tensor(t2[:], ps_f[:], c_bf2[:], h1_32[:],
                                               ALU.add, ALU.add)

                h2_16 = sb.tile([128, NL], F16, tag="hT16")
                h2_8 = None
                if li == 0:
                    h2_8 = sb.tile([128, NL], F8, tag="hT8")
                h2_32, _, _ = layer_norm(t2, h2_16, c_ln[1], c_lnn[1],
                                         t8_out=h2_8, t16_fast=(li == 1))
                return h2_32, h2_16, h2_8

            h32, h16, h8 = mha_ff_layer(0, hT32, hT16, None, hTf0)
            h32, h16, _ = mha_ff_layer(1, h32, h16, h8, None)

            # ---------- hypergraph conv ----------
            # out = relu((M @ h) @ Wh + bh), M = D^-1 H B^-1 H^T precomputed
            # on host (Dinv folded into Mt columns). Each core computes
            # pT[:, block b] = h_loc^T-partials, then one ReduceScatter sums
            # partials across cores and leaves each core its own node block.
            h_nat = sb.tile([128, 4, 128], F16, tag="h_nat")
            for c in range(4):
                pt = ps_m.tile([128, 128], F16, tag="m")
                nc.tensor.transpose(pt[:], h16[:, c * 128:(c + 1) * 128],
                                    c_id16[:])
                nc.vector.tensor_copy(h_nat[:, c, :], pt[:])
            rs_sb = sb.tile([128, W, NL], F16, tag="rs_sb")
            for b in range(W):
                pb = ps_s.tile([128, NL], F32, tag="scr", name=f"pconv{b}")
                for c in range(4):
                    nc.tensor.matmul(pb[:], h_nat[:, c, :],
                                     m_t[:, c, b * NL:(b + 1) * NL],
                                     start=(c == 0), stop=(c == 3))
                with nc.allow_low_precision(reason="conv partial fp16"):
                    if b % 2 == 1:
                        nc.vector.tensor_copy(rs_sb[:, b, :], pb[:])
                    else:
                        nc.scalar.copy(rs_sb[:, b, :], pb[:])
                nc.sync.dma_start(rs_in[b, :, :], rs_sb[:, b, :])
            nc.gpsimd.collective_compute(
                "ReduceScatter", ALU.add, replica_groups=RG,
                ins=[rs_in[:]], outs=[rs_out[:]])
            pT_loc = sb.tile([128, NL], F16, tag="pT_loc")
            nc.sync.dma_start(pT_loc[:], rs_out[:])
            po2 = ps_m.tile([128, NL], F32, tag="m")
            nc.tensor.matmul(po2[:], w_h[:], pT_loc[:])
            res = sb.tile([128, NL], F32, tag="res")
            nc.scalar.activation(res[:], po2[:], AF.Relu, bias=c_bh[:])
            nc.sync.dma_start(out_t[:], res[:])

    nc.compile()
    return nc


_NC = None


def _get_nc():
    global _NC
    if _NC is None:
        _NC = build_program()
    return _NC


def make_in_maps(inputs):
    x = np.asarray(inputs["x"], dtype=np.float32)
    edge = np.asarray(inputs["edge"])
    gw = {k: np.asarray(inputs[k], dtype=np.float32) for k in
          ("Wq", "bq", "Wk", "bk", "Wv", "bv", "Wo", "bo", "g_ln1", "b_ln1",
           "W1", "bf1", "W2", "bf2", "g_ln2", "b_ln2", "Wh", "bh")}

    node_idx = np.asarray(edge[0], dtype=np.int64)
    he_idx = np.asarray(edge[1], dtype=np.int64)
    counts = np.zeros((N, NE), dtype=np.float32)
    np.add.at(counts, (node_idx, he_idx), 1.0)
    Bdeg = counts.sum(axis=0)
    Ddeg = counts.sum(axis=1)
    Binv = np.where(Bdeg > 0, 1.0 / np.maximum(Bdeg, 1), 0.0).astype(np.float32)
    Dinv = np.where(Ddeg > 0, 1.0 / np.maximum(Ddeg, 1), 0.0).astype(np.float32)

    # S = H B^-1 H^T (symmetric, [N, N]); conv operand per core r:
    # Mt_r[jl, n] = S[loc_r(jl), n] * Dinv[n]
    from scipy import sparse
    Hs = sparse.csr_matrix(counts)
    S = np.asarray((Hs.multiply(Binv[None, :]) @ Hs.T).todense(),
                   dtype=np.float32)
    SD = S * Dinv[None, :]

    # CW16 pack: Wkv | Wq | Wo | W1G | Wh | id16 | w2(0.5) | ones/128
    cw16 = np.zeros((128, 1283), dtype=np.float16)
    cw16[:, 0:128] = gw["Wk"].astype(np.float16)
    cw16[:, 128:256] = gw["Wv"].astype(np.float16)
    cw16[:, 258:386] = gw["Wq"].astype(np.float16)
    cw16[:, 386:514] = gw["Wo"].astype(np.float16)
    w1g = 0.5 * gw["W1"] * gw["g_ln1"][:, None]
    cw16[:, 514:770] = w1g.astype(np.float16)
    cw16[:, 770:898] = gw["Wh"].astype(np.float16)
    cw16[:, 898:1026] = np.eye(128, dtype=np.float16)
    cw16[:, 1026:1282] = np.ascontiguousarray(
        (0.5 * gw["W2"]).reshape(2, 128, 128).transpose(1, 0, 2)
    ).reshape(128, 256).astype(np.float16)
    cw16[:, 1282] = 1.0 / 128

    # CW32 pack: bkv(131) | bq | bo | bf2 | bh
    cw32 = np.zeros((128, 135), dtype=np.float32)
    cw32[:, 0] = gw["bk"]
    cw32[:, 1:65] = gw["bv"][0:64][None, :]
    cw32[:, 66:130] = gw["bv"][64:128][None, :]
    cw32[:, 65] = 1.0
    cw32[:, 130] = 1.0
    cw32[:, 131] = gw["bq"]
    cw32[:, 132] = gw["bo"]
    cw32[:, 133] = gw["bf2"]
    cw32[:, 134] = gw["bh"]

    def lnrows(g, b):
        r = np.zeros((768,), dtype=np.float16)
        r[0:128] = g
        r[128:256] = b
        r[256:768] = 1.0
        return r

    # CROW pack: ln1r | ln2r | w2sum | ln1rn | ln2rn | g1n | fb1
    crow = np.zeros((1, 2562), dtype=np.float16)
    crow[0, 0:768] = lnrows(gw["g_ln1"], gw["b_ln1"])
    crow[0, 768:1536] = lnrows(gw["g_ln2"], gw["b_ln2"])
    crow[0, 1536:1664] = (0.5 * gw["W2"].sum(axis=0)).astype(np.float16)
    crow[0, 1664:1792] = (-gw["g_ln1"]).astype(np.float16)
    crow[0, 1792:1920] = (-gw["g_ln2"]).astype(np.float16)
    crow[0, 1920:2176] = (-0.5 * (gw["W1"].T @ gw["g_ln1"])).astype(np.float16)
    crow[0, 2176:2432] = (0.5 * (gw["W1"].T @ gw["b_ln1"])
                          + 0.5 * gw["bf1"]).astype(np.float16)
    crow[0, 2432:2560] = gw["bv"].astype(np.float16)

    shared = dict(CW16=cw16, CW32=cw32, CROW=crow)

    import ml_dtypes
    F8NP = ml_dtypes.float8_e4m3
    xTf = np.ascontiguousarray(
        x.T.reshape(128, W, NL)).astype(F8NP)
    in_maps = []
    for r in range(W):
        rows = slice(r * NL, (r + 1) * NL)
        xT = np.ascontiguousarray(x[rows, :].T)
        mt = np.ascontiguousarray(
            SD[rows, :].reshape(4, 128, N).transpose(1, 0, 2).astype(np.float16))
        m = dict(shared)
        m.update(xTf8=xTf, xT32=xT.astype(np.float32), xT16=xT.astype(np.float16),
                 Mt=mt)
        in_maps.append(m)
    return in_maps


def kernel(**inputs) -> np.ndarray:
    nc = _get_nc()
    in_maps = make_in_maps(inputs)
    res = run_bass_kernel_spmd(nc, in_maps, core_ids=list(range(W)))
    out = np.empty((N, D), dtype=np.float32)
    for r in range(W):
        out[r * NL:(r + 1) * NL, :] = res.results[r]["outT"].T
    return out


if __name__ == "__main__":
    build_program()
    print("build OK")

